# revision 1
# baseline (speedup 1.0000x reference)
"""Trainium2 Bass kernel for GQA causal sliding-window self-attention.

Sharding: 8 cores = 2 (batch) x 4 (KV-head groups). Each core handles one
batch element and one KV head with its 3 GQA query heads. The output
projection is computed per-group against the matching Wproj column slice;
the 4 partial outputs per batch are summed on the host.

Everything on-chip runs in feature-major ("transposed") layout so that all
matmul contractions have their contraction dim on SBUF partitions and all
DRAM traffic is contiguous. fp32r matmuls (full-rate) with fp32 PSUM
accumulation. Sliding-window/causal masking is applied by accumulating a
host-precomputed -1e9 additive mask tile into the scores PSUM via an
identity matmul (exp then underflows to exactly 0).
"""

import os
import sys
import numpy as np

sys.path.insert(0, "/opt/trn_rl_repo")

from contextlib import ExitStack

from concourse import mybir, bacc, tile
from concourse.bass_utils import run_bass_kernel_spmd

f32 = mybir.dt.float32
f32r = mybir.dt.float32r
AF = mybir.ActivationFunctionType

B, T, C = 2, 2048, 1536
H, KV, D = 12, 4, 128
REP = H // KV          # 3 query heads per kv head
QD = REP * D           # 384
VE_GATE_CH = 12
N_CORES = 8
TC = 512               # t-chunk width (matmul moving free dim)
NTC = T // TC          # 4
NCC = C // 128         # 12 contraction chunks
NST = T // 128         # 16 s-tiles

_EPS = float(np.finfo(np.float32).eps)
# all scale constants folded into the q-side rsqrt:
#   rq = (1.2*1.2/sqrt(D)) * rsqrt(mean(q^2)+eps),  rk = rsqrt(mean(k^2)+eps)
_LNCQ = float(np.log(1.2 * 1.2 / np.sqrt(D)))
_MASKVAL = -1.0e9

_CACHE = {}


def _setup_act_tables():
    """Reorder activation-table sets so ln+exp share one set (avoids ~33
    table reloads).  Patches both the bacc-side set picker and the walrus
    --act-root-json (they must agree on set indices)."""
    try:
        import json
        import tempfile
        import concourse.hw_specs as hw_specs
        import concourse.bacc as bacc_mod
        from neuronxcc.driver.Job import Job
        from neuronxcc.driver.jobs.support.FindActInfo import findActInfoFile

        src = findActInfoFile(Job.getPackageDir(), "gen3")
        if not src or not os.path.exists(src):
            return
        src_dir = os.path.dirname(src)
        dst = os.path.join(tempfile.gettempdir(), "bass_act_pwp_lnexp")
        os.makedirs(dst, exist_ok=True)
        for f in os.listdir(src_dir):
            tgt = os.path.join(dst, f)
            if not os.path.exists(tgt):
                try:
                    os.symlink(os.path.join(src_dir, f), tgt)
                except OSError:
                    pass
        d = json.load(open(src))
        sets = d["act_func_sets"]
        idx = [i for i, s in enumerate(sets)
               if s["name"] == "natural_log_exp_and_others"]
        if not idx:
            return
        sets.insert(0, sets.pop(idx[0]))
        jp = os.path.join(dst, "act_info.json")
        if os.path.lexists(jp):
            os.remove(jp)
        json.dump(d, open(jp, "w"))
        os.environ["BASS_ACT_ROOT_JSON_PATH"] = jp

        orig = hw_specs.get_activation_tables

        def reordered(arch):
            t = orig(arch)
            key = "natural_log_exp_and_others"
            if key in t:
                out = {key: t[key]}
                out.update((k, v) for k, v in t.items() if k != key)
                return out
            return t

        hw_specs.get_activation_tables = reordered
        bacc_mod.get_activation_tables = reordered
    except Exception:
        pass


_setup_act_tables()


def _partial_deltas(window, win_finite):
    """Tile-offset classes (delta = t0 - s0) that need an additive mask."""
    deltas = set()
    for dlt in range(-(TC - 128), 0 + 1, 128):        # causal partials
        deltas.add(dlt)
    if win_finite:
        dlt = window - (window % 128)                  # window partials
        while dlt + (TC - 1) > window:
            if dlt >= -(TC - 128):
                deltas.add(dlt)
            dlt -= 128
    return sorted(deltas)


def _build(window: int):
    win_finite = 0 <= window < T
    deltas = _partial_deltas(window, win_finite)
    wdeltas = [d for d in deltas if win_finite and d > window - (TC - 1)]
    wmin = min(wdeltas) if wdeltas else 0
    WIDE = TC + (TC - 128)                  # covers 4 deltas of 128
    NM = 2 if wdeltas else 1

    nc = bacc.Bacc("TRN2", target_bir_lowering=False, debug=False,
                   num_devices=N_CORES)

    xT = nc.dram_tensor("xT", [C, T], f32r, kind="ExternalInput")
    wqT = nc.dram_tensor("wqT", [C, QD], f32r, kind="ExternalInput")
    wkT = nc.dram_tensor("wkT", [C, D], f32r, kind="ExternalInput")
    wvT = nc.dram_tensor("wvT", [C, D], f32r, kind="ExternalInput")
    wpT = nc.dram_tensor("wpT", [QD, C], f32r, kind="ExternalInput")
    wg = nc.dram_tensor("wg", [VE_GATE_CH, 1], f32r, kind="ExternalInput")
    veT = nc.dram_tensor("veT", [D, T], f32r, kind="ExternalInput")
    cos2 = nc.dram_tensor("cos2", [128, T], f32r, kind="ExternalInput")
    sin2m = nc.dram_tensor("sin2m", [128, T], f32r, kind="ExternalInput")
    eye = nc.dram_tensor("eye", [128, 128], f32r, kind="ExternalInput")
    onesI = nc.dram_tensor("onesI", [128, 1], f32r, kind="ExternalInput")
    masksI = nc.dram_tensor("masksI", [NM * 128, WIDE], f32r, kind="ExternalInput")
    outT = nc.dram_tensor("outT", [C, T], f32, kind="ExternalOutput")

    with tile.TileContext(nc) as tc, ExitStack() as ctx:
        # ---- persistent SBUF pools ----
        pw = ctx.enter_context(tc.tile_pool(name="pw", bufs=1))
        pbig = ctx.enter_context(tc.tile_pool(name="pbig", bufs=1))
        prow = ctx.enter_context(tc.tile_pool(name="prow", bufs=6))
        pbc = ctx.enter_context(tc.tile_pool(name="pbc", bufs=4))

        # ---- PSUM pools (8 banks total, elastic shared tags) ----
        psAO = ctx.enter_context(tc.tile_pool(name="psAO", bufs=1, space="PSUM"))
        psR = ctx.enter_context(tc.tile_pool(name="psR", bufs=2, space="PSUM"))
        psSY = ctx.enter_context(tc.tile_pool(name="psSY", bufs=5, space="PSUM"))

        # small constants (needed from phase 1)
        wg_sb = pw.tile([VE_GATE_CH, 1], f32r, tag="wg")
        nc.sync.dma_start(wg_sb[:], wg.ap()[:])
        ones_sb = pw.tile([128, 1], f32r, tag="ones")
        nc.sync.dma_start(ones_sb[:], onesI.ap()[:])
        eye_sb = pw.tile([128, 128], f32r, tag="eye")
        nc.sync.dma_start(eye_sb[:], eye.ap()[:])
        masks_sb = pw.tile([128, NM, WIDE], f32r, tag="masks")
        eps_row = pw.tile([128, 1], f32, tag="epsr")
        nc.vector.memset(eps_row[:], _EPS)
        lncq_row = pw.tile([128, 1], f32, tag="lncq")
        nc.vector.memset(lncq_row[:], _LNCQ)

        # big persistent activations
        qT_sb = [pbig.tile([128, T], f32r, tag=f"qT{m}", name=f"qT{m}")
                 for m in range(REP)]
        kT_sb = pbig.tile([128, T], f32r, tag="kT")
        V_sb = pbig.tile([128, NST, D], f32r, tag="V")

        xT_re = xT.ap().rearrange("(cc p) t -> p cc t", p=128)

        # =========== phase 1: projections + gate + rope + rmsnorm ===========
        pending_pe = []   # deferred PE ops (sumsq matmuls, v transposes)

        def flush_pe(n=None):
            k = len(pending_pe) if n is None else min(n, len(pending_pe))
            for _ in range(k):
                pending_pe.pop(0)()

        with ExitStack() as ctx1:
            p1w = ctx1.enter_context(tc.tile_pool(name="p1w", bufs=1))
            pxt = ctx1.enter_context(tc.tile_pool(name="pxt", bufs=5))
            pcs = ctx1.enter_context(tc.tile_pool(name="pcs", bufs=2))
            ptmp = ctx1.enter_context(tc.tile_pool(name="ptmp", bufs=12))

            wk_sb = p1w.tile([128, NCC, D], f32r, tag="wk")
            wkT_re = wkT.ap().rearrange("(cc p) m -> p cc m", p=128)
            for g0 in range(0, NCC, 6):
                nc.sync.dma_start(wk_sb[:, g0:g0 + 6, :], wkT_re[:, g0:g0 + 6, :])
            wv_sb = p1w.tile([128, NCC, D], f32r, tag="wv")
            wvT_re = wvT.ap().rearrange("(cc p) m -> p cc m", p=128)

            HTC = TC // 2

            def load_xt(tci):
                eng = nc.sync
                halves = []
                for hh in range(2):
                    t0 = tci * TC + hh * HTC
                    xth = pxt.tile([128, NCC, HTC], f32r, tag="xt", name="xth")
                    for g0 in range(0, NCC, 4):
                        eng.dma_start(xth[:, g0:g0 + 4, :],
                                      xT_re[:, g0:g0 + 4, t0:t0 + HTC])
                    halves.append(xth)
                return halves

            xt_next = load_xt(0)
            for g0 in range(0, NCC, 6):
                nc.scalar.dma_start(wv_sb[:, g0:g0 + 6, :], wvT_re[:, g0:g0 + 6, :])
            wq_sb = p1w.tile([128, NCC, QD], f32r, tag="wq")
            wqT_re = wqT.ap().rearrange("(cc p) m -> p cc m", p=128)
            for g0 in range(0, NCC, 3):
                nc.sync.dma_start(wq_sb[:, g0:g0 + 3, :], wqT_re[:, g0:g0 + 3, :])

            for tci in range(NTC):
                t0 = tci * TC
                xt = xt_next
                cs = pcs.tile([128, TC], f32r, tag="cs")
                nc.sync.dma_start(cs[:], cos2.ap()[:, t0:t0 + TC])
                sn = pcs.tile([128, TC], f32r, tag="sn")
                nc.sync.dma_start(sn[:], sin2m.ap()[:, t0:t0 + TC])
                ve_t = pcs.tile([128, TC], f32r, tag="vet")
                nc.sync.dma_start(ve_t[:], veT.ap()[:, t0:t0 + TC])
                if tci + 1 < NTC:
                    xt_next = load_xt(tci + 1)

                # ve gate: sigmoid(x[:, :12] @ wg); the *3 is folded into veT
                zg = psR.tile([1, TC], f32, tag="row")
                nc.tensor.matmul(zg[0:1, 0:HTC], wg_sb[:],
                                 xt[0][0:VE_GATE_CH, 0, :],
                                 start=True, stop=False)
                nc.tensor.matmul(zg[0:1, HTC:TC], wg_sb[:],
                                 xt[1][0:VE_GATE_CH, 0, :],
                                 start=False, stop=True)
                ez = prow.tile([1, TC], f32, tag="g")
                nc.scalar.activation(ez[:], zg[:], AF.Exp, scale=-1.0)
                ez1 = prow.tile([1, TC], f32, tag="g")
                nc.vector.tensor_scalar_add(ez1[:], ez[:], 1.0)
                grow = prow.tile([1, TC], f32, tag="g")
                nc.vector.reciprocal(grow[:], ez1[:])
                gbc = pbc.tile([128, TC], f32, tag="bc")
                nc.gpsimd.partition_broadcast(gbc[:], grow[:])

                streams = [("k", 0)] + [("q", m) for m in range(REP)] + [("v", 0)]
                for kind, m in streams:
                    acc = psSY.tile([128, TC], f32, tag="sy", name="acc")
                    for hh in range(2):
                        for cc in range(NCC):
                            if kind == "q":
                                lhsT = wq_sb[:, cc, m * D:(m + 1) * D]
                            elif kind == "k":
                                lhsT = wk_sb[:, cc, :]
                            else:
                                lhsT = wv_sb[:, cc, :]
                            nc.tensor.matmul(
                                acc[:, hh * HTC:(hh + 1) * HTC], lhsT,
                                xt[hh][:, cc, :],
                                start=(cc == 0 and hh == 0),
                                stop=(cc == NCC - 1 and hh == 1))

                    if kind == "v":
                        # v += gate * ve; then transpose into natural [s, D]
                        vtmp = ptmp.tile([128, TC], f32, tag="t")
                        nc.vector.tensor_mul(vtmp[:], gbc[:], ve_t[:])
                        vfull = ptmp.tile([128, TC], f32r, tag="t")
                        nc.vector.tensor_add(vfull[:], vtmp[:], acc[:])

                        def vtrans(tci=tci, vfull=vfull):
                            for j in range(TC // 128):
                                st = tci * (TC // 128) + j
                                vtr = psSY.tile([128, 128], f32r, tag="sy",
                                                name="vtr")
                                nc.tensor.transpose(
                                    vtr[:], vfull[:, j * 128:(j + 1) * 128],
                                    eye_sb[:])
                                if j % 2 == 0:
                                    nc.scalar.copy(V_sb[:, st, :], vtr[:])
                                else:
                                    nc.vector.tensor_copy(V_sb[:, st, :], vtr[:])
                        pending_pe.append(vtrans)
                        continue

                    # q/k: evacuate PSUM early, then rmsnorm stats off SBUF
                    qraw = ptmp.tile([128, TC], f32r, tag="t")
                    nc.scalar.copy(qraw[:], acc[:])
                    sqr = ptmp.tile([128, TC], f32r, tag="t")
                    nc.scalar.activation(sqr[:], qraw[:], AF.Square)

                    def final(kind=kind, m=m, qraw=qraw, sqr=sqr, t0=t0,
                              cs=cs, sn=sn):
                        ss = psR.tile([1, TC], f32, tag="row", name="ss")
                        nc.tensor.matmul(ss[:], ones_sb[:], sqr[:],
                                         start=True, stop=True)
                        lnr = prow.tile([1, TC], f32, tag="r", name="lnr")
                        nc.scalar.activation(lnr[:], ss[:], AF.Ln,
                                             scale=1.0 / D, bias=eps_row[0:1, :])
                        rr = prow.tile([1, TC], f32, tag="r", name="rr")
                        if kind == "q":
                            nc.scalar.activation(rr[:], lnr[:], AF.Exp,
                                                 scale=-0.5,
                                                 bias=lncq_row[0:1, :])
                        else:
                            nc.scalar.activation(rr[:], lnr[:], AF.Exp,
                                                 scale=-0.5, bias=0.0)
                        rbc = pbc.tile([128, TC], f32, tag="bc", name="rbc")
                        nc.gpsimd.partition_broadcast(rbc[:], rr[:])

                        qn = ptmp.tile([128, TC], f32r, tag="t", name="qn")
                        nc.vector.tensor_mul(qn[:], rbc[:], qraw[:])
                        # rope: out = qn*[cos;cos] + swap(qn)*[sin;-sin]
                        qsw = ptmp.tile([128, TC], f32r, tag="t", name="qsw")
                        nc.sync.dma_start(qsw[0:64, :], qn[64:128, :])
                        nc.sync.dma_start(qsw[64:128, :], qn[0:64, :])
                        ta = ptmp.tile([128, TC], f32r, tag="t", name="ta")
                        nc.vector.tensor_mul(ta[:], qn[:], cs[:])
                        tb = ptmp.tile([128, TC], f32r, tag="t", name="tb")
                        nc.vector.tensor_mul(tb[:], qsw[:], sn[:])
                        dst = qT_sb[m] if kind == "q" else kT_sb
                        nc.vector.tensor_add(dst[:, t0:t0 + TC], ta[:], tb[:])
                    pending_pe.append(final)

                    # keep PE dense: flush one deferred op per stream
                    if len(pending_pe) > 1:
                        flush_pe(1)
                if tci == 1:
                    nc.scalar.dma_start(
                        masks_sb[:],
                        masksI.ap().rearrange("(nd p) t -> p nd t", p=128))
            flush_pe()

        # =========== phase 2+3 per t-chunk: attention + out-proj ===========
        pw2 = ctx.enter_context(tc.tile_pool(name="pw2", bufs=1))
        wp_sb = pw2.tile([128, REP, C], f32r, tag="wp")
        nc.scalar.dma_start(wp_sb[:], wpT.ap().rearrange("(qc p) c -> p qc c",
                                                         p=128))
        yT_sb = [pw2.tile([128, T], f32r, tag=f"yT{m}", name=f"yT{m}")
                 for m in range(REP)]
        pP = ctx.enter_context(tc.tile_pool(name="pP", bufs=6))
        pout = ctx.enter_context(tc.tile_pool(name="pout", bufs=3))

        for tci in range(NTC):
            t0 = tci * TC
            if win_finite:
                st_min = max(0, (t0 - window - 127) // 128 + 1)
            else:
                st_min = 0
            st_max = (t0 + TC - 1) // 128
            sts = list(range(st_min, st_max + 1))

            for h in range(REP):
                yU = psSY.tile([128, TC], f32, tag="sy", name="yU")
                den = psR.tile([1, TC], f32, tag="row", name="den")
                q_rhs = qT_sb[h][:, t0:t0 + TC]
                pends = []    # software-pipeline den/Y two s-tiles behind
                for idx, st in enumerate(sts):
                    s0 = st * 128
                    delta = t0 - s0
                    causal_p = delta <= 0
                    window_p = win_finite and delta > window - (TC - 1)
                    nmm = int(causal_p) + int(window_p)
                    # valid column range for this s-tile (outside it every
                    # element is masked, so P is exactly 0 there and the
                    # mask-MM covers those columns of the scores bank)
                    v0 = max(0, -delta) if causal_p else 0
                    v1 = min(TC, window - delta + 128) if window_p else TC
                    if v1 - v0 < 256:      # stay on the fp32r fast path
                        v0, v1 = 0, TC
                    sc = psSY.tile([128, TC], f32, tag="sy", name="sc")
                    nc.tensor.matmul(sc[:, v0:v1], kT_sb[:, s0:s0 + 128],
                                     qT_sb[h][:, t0 + v0:t0 + v1],
                                     start=True, stop=(nmm == 0))
                    if causal_p:    # masked cols [0, 128-delta)
                        c0, c1 = 0, max(256, min(TC, 128 - delta))
                        off = delta + (TC - 128)
                        nmm -= 1
                        nc.tensor.matmul(sc[:, c0:c1], eye_sb[:],
                                         masks_sb[:, 0, off + c0:off + c1],
                                         start=False, stop=(nmm == 0))
                    if window_p:    # masked cols suffix
                        c0 = min(TC - 256,
                                 (window - delta + 1) // 128 * 128)
                        c1 = TC
                        off = delta - wmin
                        nmm -= 1
                        nc.tensor.matmul(sc[:, c0:c1], eye_sb[:],
                                         masks_sb[:, 1, off + c0:off + c1],
                                         start=False, stop=(nmm == 0))
                    if len(pends) >= 2:
                        pends.pop(0)()
                    P = pP.tile([128, TC], f32r, tag="P", name="P")
                    nc.scalar.activation(P[:, v0:v1], sc[:, v0:v1], AF.Exp)

                    def mk(idx=idx, st=st, P=P, v0=v0, v1=v1):
                        first, last = idx == 0, idx == len(sts) - 1
                        def go():
                            nc.tensor.matmul(den[0:1, v0:v1], ones_sb[:],
                                             P[:, v0:v1],
                                             start=first, stop=last)
                            nc.tensor.matmul(yU[:, v0:v1], V_sb[:, st, :],
                                             P[:, v0:v1],
                                             start=first, stop=last)
                        return go
                    pends.append(mk())
                while pends:
                    pends.pop(0)()
                dinv = prow.tile([1, TC], f32, tag="r", name="dinv")
                nc.vector.reciprocal(dinv[:], den[:])
                dbc = pbc.tile([128, TC], f32, tag="bc", name="dbc")
                nc.gpsimd.partition_broadcast(dbc[:], dinv[:])
                nc.vector.tensor_mul(yT_sb[h][:, t0:t0 + TC], dbc[:], yU[:])

            # ---- out-proj for this t-chunk ----
            for cc in range(NCC):
                if tci == NTC - 1 and cc % 2 == 1:
                    o = psSY.tile([128, TC], f32, tag="sy", name="o")
                else:
                    o = psAO.tile([128, TC], f32, tag="ao", name="o")
                for m in range(REP):
                    nc.tensor.matmul(o[:], wp_sb[:, m, cc * 128:(cc + 1) * 128],
                                     yT_sb[m][:, t0:t0 + TC],
                                     start=(m == 0), stop=(m == REP - 1))
                ot = pout.tile([128, TC], f32, tag="ot", name="ot")
                if cc % 2 == 0:
                    nc.vector.tensor_copy(ot[:], o[:])
                else:
                    nc.scalar.copy(ot[:], o[:])
                nc.sync.dma_start(outT.ap()[cc * 128:(cc + 1) * 128,
                                            t0:t0 + TC], ot[:])

    nc.compile()
    nc._mask_cfg = {"wide": WIDE, "cmin": -(TC - 128), "wmin": wmin}
    return nc


def _prep_inputs(nc, window, x, ve, cos, sin, Wq, Wk, Wv, Wproj, Wg):
    """Build the 8 per-core input maps (host-side sharding + transposes)."""
    win_finite = 0 <= window < T
    cosT = np.ascontiguousarray(cos.reshape(T, D // 2).T)
    sinT = np.ascontiguousarray(sin.reshape(T, D // 2).T)
    cos2 = np.concatenate([cosT, cosT], axis=0)
    sin2m = np.concatenate([sinT, -sinT], axis=0)
    eye = np.eye(128, dtype=np.float32)
    ones = np.ones((128, 1), dtype=np.float32)

    ds = np.arange(128)[:, None]
    wcfg = nc._mask_cfg
    j = np.arange(wcfg["wide"])[None, :]
    mc = np.where(j + wcfg["cmin"] - ds >= 0, 0.0, _MASKVAL).astype(np.float32)
    rows = [mc]
    if win_finite:
        mw = np.where(j + wcfg["wmin"] - ds <= window, 0.0,
                      _MASKVAL).astype(np.float32)
        rows.append(mw)
    masks = np.concatenate(rows, axis=0)

    xTb = [np.ascontiguousarray(x[b].T) for b in range(B)]

    in_maps = []
    for core in range(N_CORES):
        b, g = divmod(core, KV)
        sl_q = slice(g * QD, (g + 1) * QD)
        sl_d = slice(g * D, (g + 1) * D)
        in_maps.append({
            "xT": xTb[b],
            "wqT": np.ascontiguousarray(Wq[sl_q].T),
            "wkT": np.ascontiguousarray(Wk[sl_d].T),
            "wvT": np.ascontiguousarray(Wv[sl_d].T),
            "wpT": np.ascontiguousarray(Wproj[:, sl_q].T),
            "wg": np.ascontiguousarray(Wg[g].reshape(VE_GATE_CH, 1)),
            "veT": np.ascontiguousarray(3.0 * ve[b, :, sl_d].T),
            "cos2": cos2, "sin2m": sin2m, "eye": eye, "onesI": ones,
            "masksI": masks,
        })
    return in_maps


def kernel(x, ve, cos, sin, Wq, Wk, Wv, Wproj, Wg, window, _trace=False):
    window = int(window)
    if window not in _CACHE:
        _CACHE[window] = _build(window)
    nc = _CACHE[window]

    in_maps = _prep_inputs(nc, window,
                           np.asarray(x, np.float32), np.asarray(ve, np.float32),
                           np.asarray(cos, np.float32), np.asarray(sin, np.float32),
                           np.asarray(Wq, np.float32), np.asarray(Wk, np.float32),
                           np.asarray(Wv, np.float32), np.asarray(Wproj, np.float32),
                           np.asarray(Wg, np.float32))

    res = run_bass_kernel_spmd(nc, in_maps, core_ids=list(range(N_CORES)),
                               trace=_trace)

    out = np.empty((B, T, C), dtype=np.float32)
    for b in range(B):
        acc = res.results[b * KV]["outT"].copy()
        for g in range(1, KV):
            acc += res.results[b * KV + g]["outT"]
        out[b] = acc.T
    if _trace:
        kernel._last_trace = res
    return out



# revision 28
# speedup vs baseline: 1.3532x; 1.3532x over previous
"""Trainium2 Bass kernel for GQA causal sliding-window self-attention (v2).

Sharding: 8 cores = 2 (batch) x 4 (KV-head groups). Each core handles one
batch element and one KV head with its 3 GQA query heads. The output
projection is computed per-group against the matching Wproj column slice;
the 4 partial outputs per batch are summed on the host.

v2 speedups over the fp32r baseline:
- QKV projection in fp8e4 DoubleRow (2 k-tiles/pass, 0.5 cy/row) with a
  hi/lo error-compensated split: x*W ~= xh*wh + xh16*wl16 + xl16*wh16,
  where *16/*(1/16) scale folds keep every fp8 operand in range. All three
  terms accumulate in one fp32 PSUM at scale 32; the 1/32 comes out in the
  PSUM evacuation / rmsnorm constants. ~bf16-level accuracy at 0.75x the
  fp32r PE cost.
- Everything downstream in fp16 (scores, P, V, y, Wproj): same 1.0 cy/row
  PE rate as fp32r but 2x DVE throughput, half the DMA bytes, and no
  fast-path width constraint (allows exact [v0:v1] restriction).
- Softmax denominator accumulated on the (underloaded) DVE engine instead
  of ones-matmuls on the PE; only one final [1,TC] ones-matmul per
  (head, t-chunk).
- k-side rmsnorm scale folded into the exp activation's per-partition
  scale operand (P = exp(rk[s] * sc - 4)); the -4 shift keeps exp in fp16
  range. Column-constant shifts cancel in softmax.
- Phase interleaving: projection of t-chunk i+1 is issued before the
  attention of chunk i, so the PE never drains while Act/DVE catch up.
- Evacuations and broadcasts spread across Act/DVE/Pool by load.
"""

import os
import sys
import numpy as np

sys.path.insert(0, "/opt/trn_rl_repo")

from contextlib import ExitStack

from concourse import mybir, bacc, tile
from concourse.bass_utils import run_bass_kernel_spmd

f32 = mybir.dt.float32
f16 = mybir.dt.float16
e4m3 = mybir.dt.float8e4
AF = mybir.ActivationFunctionType
PM = mybir.MatmulPerfMode

B, T, C = 2, 2048, 1536
H, KV, D = 12, 4, 128
REP = H // KV          # 3 query heads per kv head
QD = REP * D           # 384
VE_GATE_CH = 12
N_CORES = 8
TC = 512               # t-chunk width
NTC = T // TC          # 4
NCC = C // 128         # 12 contraction chunks
NPG = NCC // 2         # 6 DoubleRow pair-groups
NST = T // 128         # 16 s-tiles

_EPS = float(np.finfo(np.float32).eps)
_WS = 32.0             # weight prescale for fp8 range
_LNCQ = float(np.log(1.2 * 1.2 / np.sqrt(D)))   # q-side fold: both 1.2s + 1/sqrt(D)
_MASKVAL = -30000.0    # fits fp16; * rk then exp -> exactly 0
_PSHIFT = -4.0         # logit shift so exp fits fp16 comfortably

_CACHE = {}


def _setup_act_tables():
    """Reorder activation-table sets so ln+exp(+square+copy) share one set.
    (Same trick as baseline; sim charges 1.28us per explicit table load.)"""
    try:
        import json
        import tempfile
        import concourse.hw_specs as hw_specs
        import concourse.bacc as bacc_mod
        from neuronxcc.driver.Job import Job
        from neuronxcc.driver.jobs.support.FindActInfo import findActInfoFile

        src = findActInfoFile(Job.getPackageDir(), "gen3")
        if not src or not os.path.exists(src):
            return
        src_dir = os.path.dirname(src)
        dst = os.path.join(tempfile.gettempdir(), "bass_act_pwp_lnexp")
        os.makedirs(dst, exist_ok=True)
        for f in os.listdir(src_dir):
            tgt = os.path.join(dst, f)
            if not os.path.exists(tgt):
                try:
                    os.symlink(os.path.join(src_dir, f), tgt)
                except OSError:
                    pass
        d = json.load(open(src))
        sets = d["act_func_sets"]
        idx = [i for i, s in enumerate(sets)
               if s["name"] == "natural_log_exp_and_others"]
        if not idx:
            return
        sets.insert(0, sets.pop(idx[0]))
        jp = os.path.join(dst, "act_info.json")
        if os.path.lexists(jp):
            os.remove(jp)
        json.dump(d, open(jp, "w"))
        os.environ["BASS_ACT_ROOT_JSON_PATH"] = jp

        orig = hw_specs.get_activation_tables

        def reordered(arch):
            t = orig(arch)
            key = "natural_log_exp_and_others"
            if key in t:
                out = {key: t[key]}
                out.update((k, v) for k, v in t.items() if k != key)
                return out
            return t

        hw_specs.get_activation_tables = reordered
        bacc_mod.get_activation_tables = reordered
    except Exception:
        pass


_setup_act_tables()


def _build(window: int):
    win_finite = 0 <= window < T
    # additive-mask tile classes, same scheme as baseline
    wdeltas = []
    if win_finite:
        dlt = window - (window % 128)
        while dlt + (TC - 1) > window:
            if dlt >= -(TC - 128):
                wdeltas.append(dlt)
            dlt -= 128
    wmin = min(wdeltas) if wdeltas else 0
    WIDE = TC + (TC - 128)
    NM = 2 if wdeltas else 1

    nc = bacc.Bacc("TRN2", target_bir_lowering=False, debug=False,
                   num_devices=N_CORES)

    # --- DRAM inputs ---
    # x in hi/lo16 fp8 pairs (xi: 0=hi, 1=lo16); weights in (hi, lo_u, hi16)
    # triplets. x*W ~= xh*wh + xh*wl_u + xl16*wh16, all at PSUM scale 32.
    # chunk-major so one (tci, xi) DMA is a contiguous [128, NPG*2*TC] copy
    x3 = nc.dram_tensor("x3", [NTC, 2, 128, NPG, 2, TC], e4m3,
                        kind="ExternalInput")
    wq3 = nc.dram_tensor("wq3", [3, NPG, 128, 2, QD], e4m3, kind="ExternalInput")
    wk3 = nc.dram_tensor("wk3", [3, NPG, 128, 2, D], e4m3, kind="ExternalInput")
    wv3 = nc.dram_tensor("wv3", [3, NPG, 128, 2, D], e4m3, kind="ExternalInput")
    wpT = nc.dram_tensor("wpT", [QD, C], f16, kind="ExternalInput")
    wg = nc.dram_tensor("wg", [VE_GATE_CH, 1], e4m3, kind="ExternalInput")
    veT = nc.dram_tensor("veT", [D, T], f16, kind="ExternalInput")
    cos2 = nc.dram_tensor("cos2", [128, T], f16, kind="ExternalInput")
    sin2m = nc.dram_tensor("sin2m", [128, T], f16, kind="ExternalInput")
    eyeI = nc.dram_tensor("eyeI", [128, 128], f16, kind="ExternalInput")
    onesI = nc.dram_tensor("onesI", [128, 1], f16, kind="ExternalInput")
    masksI = nc.dram_tensor("masksI", [NM * 128, WIDE], f16, kind="ExternalInput")
    outT = nc.dram_tensor("outT", [C, T], f16, kind="ExternalOutput")

    with tile.TileContext(nc) as tc, ExitStack() as ctx:
        # ---- persistent SBUF pools ----
        pw = ctx.enter_context(tc.tile_pool(name="pw", bufs=1))
        pbig = ctx.enter_context(tc.tile_pool(name="pbig", bufs=1))
        pxt = ctx.enter_context(tc.tile_pool(name="pxt", bufs=2))
        pcs = ctx.enter_context(tc.tile_pool(name="pcs", bufs=2))
        prow = ctx.enter_context(tc.tile_pool(name="prow", bufs=3))
        pbc = ctx.enter_context(tc.tile_pool(name="pbc", bufs=3))
        pqn = ctx.enter_context(tc.tile_pool(name="pqn", bufs=2))
        ptmp = ctx.enter_context(tc.tile_pool(name="ptmp", bufs=8))
        pP = ctx.enter_context(tc.tile_pool(name="pP", bufs=4))
        pden = ctx.enter_context(tc.tile_pool(name="pden", bufs=2))
        pout = ctx.enter_context(tc.tile_pool(name="pout", bufs=3))

        # ---- PSUM pools (8 banks) ----
        pacc = ctx.enter_context(tc.tile_pool(name="pacc", bufs=2, space="PSUM"))
        psR = ctx.enter_context(tc.tile_pool(name="psR", bufs=2, space="PSUM"))
        psSY = ctx.enter_context(tc.tile_pool(name="psSY", bufs=4, space="PSUM"))

        # ---- tiny constants first (DMA device is serial: order matters) ----
        wg_sb = pw.tile([VE_GATE_CH, 1], e4m3, tag="wg")
        nc.sync.dma_start(wg_sb[:], wg.ap()[:])
        ones_sb = pw.tile([128, 1], f16, tag="ones")
        nc.sync.dma_start(ones_sb[:], onesI.ap()[:])
        eps_row = pw.tile([128, 1], f32, tag="epsr")
        nc.vector.memset(eps_row[:], _EPS)
        lncq_row = pw.tile([128, 1], f32, tag="lncq")
        nc.vector.memset(lncq_row[:], _LNCQ)
        pshift_col = pw.tile([128, 1], f32, tag="pshift")
        nc.vector.memset(pshift_col[:], _PSHIFT)

        # ---- persistent activations ----
        qT_sb = [pbig.tile([128, T], f16, tag=f"qT{m}", name=f"qT{m}")
                 for m in range(REP)]
        kT_sb = pbig.tile([128, T], f16, tag="kT")
        V_sb = pbig.tile([128, NST, D], f16, tag="V")
        rk_cols = pbig.tile([128, NST], f32, tag="rkc")
        yT_sb = [pbig.tile([128, T], f16, tag=f"yT{m}", name=f"yT{m}")
                 for m in range(REP)]

        x_re = x3.ap().rearrange("tc xi p g j t -> tc xi p g j t")

        def load_x(tci):
            xt = pxt.tile([128, 2, NPG, 2, TC], e4m3, tag="xt", name=f"x{tci}")
            for xi in range(2):
                nc.sync.dma_start(xt[:, xi], x_re[tci, xi])
            return xt

        def load_csv(tci):
            t0 = tci * TC
            cs = pcs.tile([128, TC], f16, tag="cs")
            nc.sync.dma_start(cs[:], cos2.ap()[:, t0:t0 + TC])
            sn = pcs.tile([128, TC], f16, tag="sn")
            nc.sync.dma_start(sn[:], sin2m.ap()[:, t0:t0 + TC])
            ve_t = pcs.tile([128, TC], f16, tag="vet")
            nc.sync.dma_start(ve_t[:], veT.ap()[:, t0:t0 + TC])
            return cs, sn, ve_t

        # ---- PE p-state warm-up: keep the PE busy on junk matmuls while the
        # first x chunk streams in, so real matmuls start at full clock ----
        junk = pw.tile([128, TC], f16, tag="junk")
        nc.vector.memset(junk[:], 1.0)
        for _ in range(8):
            wu = psSY.tile([128, TC], f32, tag="sy", name="wu")
            nc.tensor.matmul(wu[:], junk[:, 0:128], junk[:],
                             start=True, stop=True)

        # ---- first x chunk + weights, interleaved by first-use order ----
        xt0 = load_x(0)
        wk_sb = pw.tile([128, 3, NPG, 2, D], e4m3, tag="wk")
        nc.sync.dma_start(wk_sb[:], wk3.ap().rearrange("ti g p j m -> p ti g j m"))
        csv0 = load_csv(0)
        wq_sb = pw.tile([128, 3, NPG, 2, QD], e4m3, tag="wq")
        wq_re = wq3.ap().rearrange("ti g p j m -> p ti g j m")
        for ti in range(3):
            nc.sync.dma_start(wq_sb[:, ti], wq_re[:, ti])
        wv_sb = pw.tile([128, 3, NPG, 2, D], e4m3, tag="wv")
        nc.sync.dma_start(wv_sb[:], wv3.ap().rearrange("ti g p j m -> p ti g j m"))
        eye_sb = pw.tile([128, 128], f16, tag="eye")
        nc.sync.dma_start(eye_sb[:], eyeI.ap()[:])
        masks_sb = pw.tile([128, NM, WIDE], f16, tag="masks")
        nc.scalar.dma_start(masks_sb[:],
                            masksI.ap().rearrange("(nd p) t -> p nd t", p=128))
        wp_sb = pw.tile([128, REP, C], f16, tag="wp")
        nc.scalar.dma_start(
            wp_sb[:], wpT.ap().rearrange("(qc p) c -> p qc c", p=128))

        # round-robin evacuation helper (spread PSUM->SBUF copies)
        _evac_rr = [0]

        def evac(dst, src, engines=("act", "dve", "pool")):
            e = engines[_evac_rr[0] % len(engines)]
            _evac_rr[0] += 1
            if e == "act":
                nc.scalar.copy(dst, src)
            elif e == "dve":
                nc.vector.tensor_copy(dst, src)
            else:
                nc.gpsimd.tensor_copy(dst, src)

        # ================= phase 1: one t-chunk of projections =================
        # terms: (x index, w index): xh*wh + xh*wl_u + xl16*wh16
        TERMS = [(0, 0), (0, 1), (1, 2)]

        def p1_parts(tci, xt, csv, prefetch=None):
            """Return a list of thunks emitting one t-chunk of phase 1; they
            are interleaved between the previous chunk's attention heads."""
            t0 = tci * TC
            cs, sn, ve_t = csv
            state = {}

            def do_proj(kind, m):
                acc = pacc.tile([128, TC], f32, tag="acc", name="acc")
                w_sb = {"q": wq_sb, "k": wk_sb, "v": wv_sb}[kind]
                msl = slice(m * D, (m + 1) * D) if kind == "q" else slice(0, D)
                for i, (xi, wi) in enumerate(TERMS):
                    for g in range(NPG):
                        nc.tensor.matmul(
                            acc[:], w_sb[:, wi, g, :, msl], xt[:, xi, g, :, :],
                            start=(i == 0 and g == 0),
                            stop=(i == 2 and g == NPG - 1),
                            perf_mode=PM.DoubleRow)
                return acc

            def do_qk(kind, m, acc):
                qn_all = state["qn"]
                qraw = ptmp.tile([128, TC], f16, tag="t", name="qraw")
                nc.scalar.activation(qraw[:], acc[:], AF.Copy, scale=1.0 / _WS)
                sqr = ptmp.tile([128, TC], f16, tag="t", name="sqr")
                nc.vector.tensor_mul(sqr[:], qraw[:], qraw[:])
                if kind == "k":
                    # transposed sumsq -> rk as columns, folded into exp scale
                    rkT = psR.tile([128, 4], f32, tag="row", name="rkT")
                    for j in range(TC // 128):
                        nc.tensor.matmul(rkT[:, j:j + 1],
                                         sqr[:, j * 128:(j + 1) * 128],
                                         ones_sb[:], start=True, stop=True)
                    lncol = ptmp.tile([128, 4], f32, tag="rc", name="lncol")
                    nc.scalar.activation(lncol[:], rkT[:], AF.Ln,
                                         scale=1.0 / D, bias=eps_row[:, 0:1])
                    nc.scalar.activation(rk_cols[:, tci * 4:tci * 4 + 4],
                                         lncol[:], AF.Exp, scale=-0.5)
                    nc.vector.tensor_copy(qn_all[:, REP, :], qraw[:])
                else:
                    ss = psR.tile([1, TC], f32, tag="row", name="ss")
                    nc.tensor.matmul(ss[:], ones_sb[:], sqr[:],
                                     start=True, stop=True)
                    lnr = prow.tile([1, TC], f32, tag="r", name="lnr")
                    nc.scalar.activation(lnr[:], ss[:], AF.Ln,
                                         scale=1.0 / D, bias=eps_row[0:1, :])
                    rr = prow.tile([1, TC], f16, tag="rh", name="rr")
                    nc.scalar.activation(rr[:], lnr[:], AF.Exp,
                                         scale=-0.5, bias=lncq_row[0:1, :])
                    rbc = pbc.tile([128, TC], f16, tag="bc", name="rbc")
                    nc.gpsimd.partition_broadcast(rbc[:], rr[:])
                    nc.vector.tensor_mul(qn_all[:, m, :], rbc[:], qraw[:])

            def part_gate_k():
                if prefetch is not None:
                    prefetch()
                state["qn"] = pqn.tile([128, 4, TC], f16, tag="qn", name="qn")
                state["qsw"] = pqn.tile([128, 4, TC], f16, tag="qsw", name="qsw")
                zg = psR.tile([1, TC], f32, tag="row", name="zg")
                nc.tensor.matmul(zg[0:1, :], wg_sb[:],
                                 xt[0:VE_GATE_CH, 0, 0, 0, :],
                                 start=True, stop=True)
                ez = prow.tile([1, TC], f32, tag="g")
                nc.scalar.activation(ez[:], zg[:], AF.Exp, scale=-1.0 / 64.0)
                ez1 = prow.tile([1, TC], f32, tag="g")
                nc.vector.tensor_scalar_add(ez1[:], ez[:], 1.0)
                grow = prow.tile([1, TC], f16, tag="gh")
                nc.vector.reciprocal(grow[:], ez1[:])
                gbc = pbc.tile([128, TC], f16, tag="bc", name="gbc")
                nc.gpsimd.partition_broadcast(gbc[:], grow[:])
                state["gbc"] = gbc
                do_qk("k", 0, do_proj("k", 0))

            def part_q(m):
                def go():
                    do_qk("q", m, do_proj("q", m))
                return go

            def part_v():
                acc = do_proj("v", 0)
                vtmp = ptmp.tile([128, TC], f16, tag="t", name="vtmp")
                nc.vector.tensor_mul(vtmp[:], state["gbc"][:], ve_t[:])
                vfull = ptmp.tile([128, TC], f16, tag="t", name="vfull")
                nc.vector.scalar_tensor_tensor(
                    vfull[:], acc[:], 1.0 / _WS, vtmp[:],
                    mybir.AluOpType.mult, mybir.AluOpType.add)
                for j in range(TC // 128):
                    st = tci * (TC // 128) + j
                    vtr = psSY.tile([128, 128], f16, tag="sy", name="vtr")
                    nc.tensor.transpose(
                        vtr[:], vfull[:, j * 128:(j + 1) * 128], eye_sb[:])
                    evac(V_sb[:, st, :], vtr[:], ("act", "dve"))

            def part_rope():
                qn_all, qsw_all = state["qn"], state["qsw"]
                nc.sync.dma_start(qsw_all[0:64], qn_all[64:128])
                nc.sync.dma_start(qsw_all[64:128], qn_all[0:64])
                for m in range(REP + 1):
                    dst = qT_sb[m] if m < REP else kT_sb
                    ta = ptmp.tile([128, TC], f16, tag="t", name="ta")
                    nc.vector.tensor_mul(ta[:], qn_all[:, m, :], cs[:])
                    tb = ptmp.tile([128, TC], f16, tag="t", name="tb")
                    nc.vector.tensor_mul(tb[:], qsw_all[:, m, :], sn[:])
                    nc.vector.tensor_add(dst[:, t0:t0 + TC], ta[:], tb[:])

            return [part_gate_k, part_q(0), part_q(1),
                    lambda: (part_q(2)(), part_v()), part_rope]

        # ================= phase 2: attention + out-proj per chunk =============
        def attn_chunk(tci, fillers=()):
            t0 = tci * TC
            if win_finite:
                st_min = max(0, (t0 - window - 127) // 128 + 1)
            else:
                st_min = 0
            st_max = (t0 + TC - 1) // 128
            sts = list(range(st_min, st_max + 1))
            fillers = list(fillers)

            for h in range(REP):
                if fillers:
                    fillers.pop(0)()
                yU = psSY.tile([128, TC], f32, tag="sy", name="yU")
                den = pden.tile([128, TC], f16, tag="d", name="den")
                for idx, st in enumerate(sts):
                    s0 = st * 128
                    delta = t0 - s0
                    causal_p = delta <= 0
                    window_p = win_finite and delta > window - (TC - 1)
                    v0 = max(0, -delta) if causal_p else 0
                    v1 = min(TC, window - delta + 128) if window_p else TC
                    if idx == 0:
                        # full width so the yU start=True zeroes the whole
                        # bank; mask matmuls cover the out-of-range columns
                        # (the mask tiles encode the exact condition)
                        v0, v1 = 0, TC
                    nmm = int(causal_p) + int(window_p)
                    sc = psSY.tile([128, TC], f32, tag="sy", name="sc")
                    nc.tensor.matmul(sc[:, v0:v1], kT_sb[:, s0:s0 + 128],
                                     qT_sb[h][:, t0 + v0:t0 + v1],
                                     start=True, stop=(nmm == 0))
                    if causal_p:    # masked cols [v0, 128-delta)
                        c0, c1 = v0, min(v1, 128 - delta)
                        off = delta + (TC - 128)
                        nmm -= 1
                        nc.tensor.matmul(sc[:, c0:c1], eye_sb[:],
                                         masks_sb[:, 0, off + c0:off + c1],
                                         start=False, stop=(nmm == 0))
                    if window_p:    # masked cols suffix
                        c0 = max(v0, (window - delta + 1) // 128 * 128)
                        c1 = v1
                        off = delta - wmin
                        nmm -= 1
                        nc.tensor.matmul(sc[:, c0:c1], eye_sb[:],
                                         masks_sb[:, 1, off + c0:off + c1],
                                         start=False, stop=(nmm == 0))
                    P = pP.tile([128, TC], f16, tag="P", name="P")
                    nc.scalar.activation(P[:, v0:v1], sc[:, v0:v1], AF.Exp,
                                         scale=rk_cols[:, st:st + 1],
                                         bias=pshift_col[:, 0:1])
                    if idx == 0:    # full width: replaces a memset
                        nc.vector.tensor_copy(den[:], P[:])
                    else:
                        nc.vector.tensor_add(den[:, v0:v1], den[:, v0:v1],
                                             P[:, v0:v1])
                    nc.tensor.matmul(yU[:, v0:v1], V_sb[:, st, :],
                                     P[:, v0:v1], start=(idx == 0),
                                     stop=(idx == len(sts) - 1))
                drow = psR.tile([1, TC], f32, tag="row", name="dr")
                nc.tensor.matmul(drow[:], ones_sb[:], den[:],
                                 start=True, stop=True)
                dinv = prow.tile([1, TC], f16, tag="rh", name="dinv")
                nc.vector.reciprocal(dinv[:], drow[:])
                dbc = pbc.tile([128, TC], f16, tag="bc", name="dbc")
                nc.gpsimd.partition_broadcast(dbc[:], dinv[:])
                nc.vector.tensor_mul(yT_sb[h][:, t0:t0 + TC], dbc[:], yU[:])

            # ---- out-proj for this chunk ----
            for half in range(2):
                if fillers:
                    fillers.pop(0)()
                ot = pout.tile([128, NCC // 2, TC], f16, tag="ot", name="ot")
                for ci in range(NCC // 2):
                    cc = half * (NCC // 2) + ci
                    o = psSY.tile([128, TC], f32, tag="sy", name="o")
                    for m in range(REP):
                        nc.tensor.matmul(
                            o[:], wp_sb[:, m, cc * 128:(cc + 1) * 128],
                            yT_sb[m][:, t0:t0 + TC],
                            start=(m == 0), stop=(m == REP - 1))
                    evac(ot[:, ci, :], o[:], ("act", "dve"))
                nc.sync.dma_start(
                    outT.ap().rearrange("(cc p) t -> p cc t", p=128)
                    [:, half * 6:half * 6 + 6, t0:t0 + TC], ot[:])
            for f in fillers:
                f()

        # ================= interleaved schedule =================
        with nc.allow_low_precision(reason="fp16 validated vs numpy: 2e-3 max rel"):
            xt1 = load_x(1)
            csv1 = load_csv(1)
            xts = {0: xt0, 1: xt1}
            csvs = {0: csv0, 1: csv1}
            for p in p1_parts(0, xt0, csv0):
                p()
            for tci in range(NTC):
                fillers = ()
                if tci + 1 < NTC:
                    prefetch = None
                    if tci + 2 < NTC:
                        def prefetch(t2=tci + 2):
                            xts[t2] = load_x(t2)
                            csvs[t2] = load_csv(t2)
                    fillers = p1_parts(tci + 1, xts[tci + 1], csvs[tci + 1],
                                       prefetch)
                attn_chunk(tci, fillers)

    nc.compile()
    nc._mask_cfg = {"wide": WIDE, "cmin": -(TC - 128), "wmin": wmin}
    return nc


def _qx_hilo(a):
    """x fp8 pair: a ~= hi + lo16/16."""
    import ml_dtypes
    hi = a.astype(ml_dtypes.float8_e4m3fn)
    hif = hi.astype(np.float32)
    lo16 = ((a - hif) * 16.0).astype(ml_dtypes.float8_e4m3fn)
    return hi, lo16


def _qw_tri(a):
    """weight fp8 triplet (prescaled): hi, lo_unscaled, hi/16."""
    import ml_dtypes
    hi = a.astype(ml_dtypes.float8_e4m3fn)
    hif = hi.astype(np.float32)
    lo_u = (a - hif).astype(ml_dtypes.float8_e4m3fn)
    hi16 = (hif / 16.0).astype(ml_dtypes.float8_e4m3fn)
    return hi, lo_u, hi16


def _pack_pairs(a3, outdim_last):
    """[3][rows(C), cols] -> [3, NPG, 128, 2, cols] pair-grouped layout."""
    out = np.stack([t.reshape(NPG, 2, 128, -1).transpose(0, 2, 1, 3)
                    for t in a3], axis=0)
    return np.ascontiguousarray(out)


def _prep_inputs(nc, window, x, ve, cos, sin, Wq, Wk, Wv, Wproj, Wg):
    import ml_dtypes
    f16t = ml_dtypes.float16 if hasattr(ml_dtypes, 'float16') else np.float16
    win_finite = 0 <= window < T
    cosT = np.ascontiguousarray(cos.reshape(T, D // 2).T)
    sinT = np.ascontiguousarray(sin.reshape(T, D // 2).T)
    cos2 = np.concatenate([cosT, cosT], axis=0).astype(np.float16)
    sin2m = np.concatenate([sinT, -sinT], axis=0).astype(np.float16)
    eye = np.eye(128, dtype=np.float16)
    ones = np.ones((128, 1), dtype=np.float16)

    ds = np.arange(128)[:, None]
    wcfg = nc._mask_cfg
    j = np.arange(wcfg["wide"])[None, :]
    mc = np.where(j + wcfg["cmin"] - ds >= 0, 0.0, _MASKVAL).astype(np.float16)
    rows = [mc]
    if win_finite:
        mw = np.where(j + wcfg["wmin"] - ds <= window, 0.0,
                      _MASKVAL).astype(np.float16)
        rows.append(mw)
    masks = np.concatenate(rows, axis=0)

    # x pairs per batch, chunk-major: [NTC, 2, 128, NPG, 2, TC]
    x3b = []
    for b in range(B):
        xT = x[b].T                                   # [C, T]
        tri = np.stack(_qx_hilo(xT), axis=0)          # [2, C, T]
        tri = tri.reshape(2, NPG, 2, 128, T).transpose(0, 3, 1, 2, 4)
        chunks = [tri[:, :, :, :, t0:t0 + TC] for t0 in range(0, T, TC)]
        x3b.append(np.ascontiguousarray(np.stack(chunks, axis=0)))

    in_maps = []
    for core in range(N_CORES):
        b, g = divmod(core, KV)
        sl_q = slice(g * QD, (g + 1) * QD)
        sl_d = slice(g * D, (g + 1) * D)
        wq_t = _pack_pairs(list(_qw_tri(Wq[sl_q].T * _WS)), QD)
        wk_t = _pack_pairs(list(_qw_tri(Wk[sl_d].T * _WS)), D)
        wv_t = _pack_pairs(list(_qw_tri(Wv[sl_d].T * _WS)), D)
        in_maps.append({
            "x3": x3b[b],
            "wq3": wq_t, "wk3": wk_t, "wv3": wv_t,
            "wpT": np.ascontiguousarray(Wproj[:, sl_q].T).astype(np.float16),
            "wg": np.ascontiguousarray(
                (Wg[g] * 64.0).reshape(VE_GATE_CH, 1)).astype(
                    __import__('ml_dtypes').float8_e4m3fn),
            "veT": np.ascontiguousarray(3.0 * ve[b, :, sl_d].T).astype(np.float16),
            "cos2": cos2, "sin2m": sin2m, "eyeI": eye, "onesI": ones,
            "masksI": masks,
        })
    return in_maps


def kernel(x, ve, cos, sin, Wq, Wk, Wv, Wproj, Wg, window, _trace=False):
    window = int(window)
    if window not in _CACHE:
        _CACHE[window] = _build(window)
    nc = _CACHE[window]

    in_maps = _prep_inputs(nc, window,
                           np.asarray(x, np.float32), np.asarray(ve, np.float32),
                           np.asarray(cos, np.float32), np.asarray(sin, np.float32),
                           np.asarray(Wq, np.float32), np.asarray(Wk, np.float32),
                           np.asarray(Wv, np.float32), np.asarray(Wproj, np.float32),
                           np.asarray(Wg, np.float32))

    res = run_bass_kernel_spmd(nc, in_maps, core_ids=list(range(N_CORES)),
                               trace=_trace)

    out = np.empty((B, T, C), dtype=np.float32)
    for b in range(B):
        acc = res.results[b * KV]["outT"].astype(np.float32)
        for g in range(1, KV):
            acc += res.results[b * KV + g]["outT"].astype(np.float32)
        out[b] = acc.T
    if _trace:
        kernel._last_trace = res
    return out


# revision 41
# speedup vs baseline: 1.4122x; 1.0436x over previous
"""Trainium2 Bass kernel for GQA causal sliding-window self-attention (v2).

Sharding: 8 cores = 2 (batch) x 4 (KV-head groups). Each core handles one
batch element and one KV head with its 3 GQA query heads. The output
projection is computed per-group against the matching Wproj column slice;
the 4 partial outputs per batch are summed on the host.

v2 speedups over the fp32r baseline:
- QKV projection in fp8e4 DoubleRow (2 k-tiles/pass, 0.5 cy/row) with a
  hi/lo error-compensated split: x*W ~= xh*wh + xh16*wl16 + xl16*wh16,
  where *16/*(1/16) scale folds keep every fp8 operand in range. All three
  terms accumulate in one fp32 PSUM at scale 32; the 1/32 comes out in the
  PSUM evacuation / rmsnorm constants. ~bf16-level accuracy at 0.75x the
  fp32r PE cost.
- Everything downstream in fp16 (scores, P, V, y, Wproj): same 1.0 cy/row
  PE rate as fp32r but 2x DVE throughput, half the DMA bytes, and no
  fast-path width constraint (allows exact [v0:v1] restriction).
- Softmax denominator accumulated on the (underloaded) DVE engine instead
  of ones-matmuls on the PE; only one final [1,TC] ones-matmul per
  (head, t-chunk).
- k-side rmsnorm scale folded into the exp activation's per-partition
  scale operand (P = exp(rk[s] * sc - 4)); the -4 shift keeps exp in fp16
  range. Column-constant shifts cancel in softmax.
- Phase interleaving: projection of t-chunk i+1 is issued before the
  attention of chunk i, so the PE never drains while Act/DVE catch up.
- Evacuations and broadcasts spread across Act/DVE/Pool by load.
"""

import os
import sys
import numpy as np

sys.path.insert(0, "/opt/trn_rl_repo")

from contextlib import ExitStack

from concourse import mybir, bacc, tile
from concourse.bass_utils import run_bass_kernel_spmd

f32 = mybir.dt.float32
f16 = mybir.dt.float16
e4m3 = mybir.dt.float8e4
AF = mybir.ActivationFunctionType
PM = mybir.MatmulPerfMode

B, T, C = 2, 2048, 1536
H, KV, D = 12, 4, 128
REP = H // KV          # 3 query heads per kv head
QD = REP * D           # 384
VE_GATE_CH = 12
N_CORES = 8
TC = 512               # t-chunk width
NTC = T // TC          # 4
NCC = C // 128         # 12 contraction chunks
NPG = NCC // 2         # 6 DoubleRow pair-groups
NST = T // 128         # 16 s-tiles

_EPS = float(np.finfo(np.float32).eps)
_WS = 32.0             # weight prescale for fp8 range
_LNCQ = float(np.log(1.2 * 1.2 / np.sqrt(D)))   # q-side fold: both 1.2s + 1/sqrt(D)
_MASKVAL = -30000.0    # fits fp16; * rk then exp -> exactly 0
_PSHIFT = -4.0         # logit shift so exp fits fp16 comfortably

_CACHE = {}


def _setup_act_tables():
    """Reorder activation-table sets so ln+exp(+square+copy) share one set.
    (Same trick as baseline; sim charges 1.28us per explicit table load.)"""
    try:
        import json
        import tempfile
        import concourse.hw_specs as hw_specs
        import concourse.bacc as bacc_mod
        from neuronxcc.driver.Job import Job
        from neuronxcc.driver.jobs.support.FindActInfo import findActInfoFile

        src = findActInfoFile(Job.getPackageDir(), "gen3")
        if not src or not os.path.exists(src):
            return
        src_dir = os.path.dirname(src)
        dst = os.path.join(tempfile.gettempdir(), "bass_act_pwp_lnexp")
        os.makedirs(dst, exist_ok=True)
        for f in os.listdir(src_dir):
            tgt = os.path.join(dst, f)
            if not os.path.exists(tgt):
                try:
                    os.symlink(os.path.join(src_dir, f), tgt)
                except OSError:
                    pass
        d = json.load(open(src))
        sets = d["act_func_sets"]
        idx = [i for i, s in enumerate(sets)
               if s["name"] == "natural_log_exp_and_others"]
        if not idx:
            return
        sets.insert(0, sets.pop(idx[0]))
        jp = os.path.join(dst, "act_info.json")
        if os.path.lexists(jp):
            os.remove(jp)
        json.dump(d, open(jp, "w"))
        os.environ["BASS_ACT_ROOT_JSON_PATH"] = jp

        orig = hw_specs.get_activation_tables

        def reordered(arch):
            t = orig(arch)
            key = "natural_log_exp_and_others"
            if key in t:
                out = {key: t[key]}
                out.update((k, v) for k, v in t.items() if k != key)
                return out
            return t

        hw_specs.get_activation_tables = reordered
        bacc_mod.get_activation_tables = reordered
    except Exception:
        pass


_setup_act_tables()


def _build(window: int):
    win_finite = 0 <= window < T
    # additive-mask tile classes, same scheme as baseline
    wdeltas = []
    if win_finite:
        dlt = window - (window % 128)
        while dlt + (TC - 1) > window:
            if dlt >= -(TC - 128):
                wdeltas.append(dlt)
            dlt -= 128
    wmin = min(wdeltas) if wdeltas else 0
    WIDE = TC + (TC - 128)
    NM = 2 if wdeltas else 1

    nc = bacc.Bacc("TRN2", target_bir_lowering=False, debug=False,
                   num_devices=N_CORES)

    # --- DRAM inputs ---
    # x in hi/lo16 fp8 pairs (xi: 0=hi, 1=lo16); weights in (hi, lo_u, hi16)
    # triplets. x*W ~= xh*wh + xh*wl_u + xl16*wh16, all at PSUM scale 32.
    # chunk-major so one (tci, xi) DMA is a contiguous [128, NPG*2*TC] copy
    x3 = nc.dram_tensor("x3", [NTC, 2, 128, NPG, 2, TC], e4m3,
                        kind="ExternalInput")
    wq3 = nc.dram_tensor("wq3", [3, NPG, 128, 2, QD], e4m3, kind="ExternalInput")
    wk3 = nc.dram_tensor("wk3", [3, NPG, 128, 2, D], e4m3, kind="ExternalInput")
    wv3 = nc.dram_tensor("wv3", [3, NPG, 128, 2, D], e4m3, kind="ExternalInput")
    wpT = nc.dram_tensor("wpT", [QD, C], f16, kind="ExternalInput")
    wg = nc.dram_tensor("wg", [VE_GATE_CH, 1], e4m3, kind="ExternalInput")
    veT = nc.dram_tensor("veT", [D, T], f16, kind="ExternalInput")
    cos2 = nc.dram_tensor("cos2", [128, T], f16, kind="ExternalInput")
    sin2m = nc.dram_tensor("sin2m", [128, T], f16, kind="ExternalInput")
    eyeI = nc.dram_tensor("eyeI", [128, 128], f16, kind="ExternalInput")
    onesI = nc.dram_tensor("onesI", [128, 1], f16, kind="ExternalInput")
    masksI = nc.dram_tensor("masksI", [NM * 128, WIDE], f16, kind="ExternalInput")
    outT = nc.dram_tensor("outT", [C, T], f16, kind="ExternalOutput")

    with tile.TileContext(nc) as tc, ExitStack() as ctx:
        # ---- persistent SBUF pools ----
        pw = ctx.enter_context(tc.tile_pool(name="pw", bufs=1))
        pbig = ctx.enter_context(tc.tile_pool(name="pbig", bufs=1))
        pxt = ctx.enter_context(tc.tile_pool(name="pxt", bufs=2))
        pcs = ctx.enter_context(tc.tile_pool(name="pcs", bufs=2))
        prow = ctx.enter_context(tc.tile_pool(name="prow", bufs=3))
        pbc = ctx.enter_context(tc.tile_pool(name="pbc", bufs=3))
        pqn = ctx.enter_context(tc.tile_pool(name="pqn", bufs=2))
        ptmp = ctx.enter_context(tc.tile_pool(name="ptmp", bufs=8))
        pP = ctx.enter_context(tc.tile_pool(name="pP", bufs=6))
        pden = ctx.enter_context(tc.tile_pool(name="pden", bufs=3))
        pout = ctx.enter_context(tc.tile_pool(name="pout", bufs=3))

        # ---- PSUM pools (8 banks) ----
        pacc = ctx.enter_context(tc.tile_pool(name="pacc", bufs=2, space="PSUM"))
        psR = ctx.enter_context(tc.tile_pool(name="psR", bufs=2, space="PSUM"))
        psSY = ctx.enter_context(tc.tile_pool(name="psSY", bufs=4, space="PSUM"))

        # ---- tiny constants first (DMA device is serial: order matters) ----
        wg_sb = pw.tile([VE_GATE_CH, 1], e4m3, tag="wg")
        nc.sync.dma_start(wg_sb[:], wg.ap()[:])
        ones_sb = pw.tile([128, 1], f16, tag="ones")
        nc.sync.dma_start(ones_sb[:], onesI.ap()[:])
        eps_row = pw.tile([128, 1], f32, tag="epsr")
        nc.vector.memset(eps_row[:], _EPS)
        lncq_row = pw.tile([128, 1], f32, tag="lncq")
        nc.vector.memset(lncq_row[:], _LNCQ)
        pshift_col = pw.tile([128, 1], f32, tag="pshift")
        nc.vector.memset(pshift_col[:], _PSHIFT)

        # ---- persistent activations ----
        qT_sb = [pbig.tile([128, T], f16, tag=f"qT{m}", name=f"qT{m}")
                 for m in range(REP)]
        kT_sb = pbig.tile([128, T], f16, tag="kT")
        V_sb = pbig.tile([128, NST, D], f16, tag="V")
        rk_cols = pbig.tile([128, NST], f32, tag="rkc")
        yT_sb = [pbig.tile([128, T], f16, tag=f"yT{m}", name=f"yT{m}")
                 for m in range(REP)]

        x_re = x3.ap().rearrange("tc xi p g j t -> tc xi p g j t")

        def load_x(tci):
            xt = pxt.tile([128, 2, NPG, 2, TC], e4m3, tag="xt", name=f"x{tci}")
            for xi in range(2):
                nc.sync.dma_start(xt[:, xi], x_re[tci, xi])
            return xt

        def load_csv(tci):
            t0 = tci * TC
            cs = pcs.tile([128, TC], f16, tag="cs")
            nc.sync.dma_start(cs[:], cos2.ap()[:, t0:t0 + TC])
            sn = pcs.tile([128, TC], f16, tag="sn")
            nc.sync.dma_start(sn[:], sin2m.ap()[:, t0:t0 + TC])
            ve_t = pcs.tile([128, TC], f16, tag="vet")
            nc.sync.dma_start(ve_t[:], veT.ap()[:, t0:t0 + TC])
            return cs, sn, ve_t

        # ---- PE p-state warm-up: keep the PE busy on junk matmuls while the
        # first x chunk streams in, so real matmuls start at full clock ----
        junk = pw.tile([128, TC], f16, tag="junk")
        nc.vector.memset(junk[:], 1.0)
        for _ in range(14):
            wu = psSY.tile([128, TC], f32, tag="sy", name="wu")
            nc.tensor.matmul(wu[:], junk[:, 0:128], junk[:],
                             start=True, stop=True)

        # ---- first x chunk + weights, in strict first-use order (the DMA
        # device is serial in the cost model, so issue order = arrival order)
        xt0 = pxt.tile([128, 2, NPG, 2, TC], e4m3, tag="xt", name="x0")
        nc.sync.dma_start(xt0[:, 0], x3.ap().rearrange(
            "tc xi p g j t -> tc xi p g j t")[0, 0])
        wk_sb = pw.tile([128, 3, NPG, 2, D], e4m3, tag="wk")
        wk_re = wk3.ap().rearrange("ti g p j m -> p ti g j m")
        nc.sync.dma_start(wk_sb[:, 0:2], wk_re[:, 0:2])
        nc.sync.dma_start(xt0[:, 1], x3.ap().rearrange(
            "tc xi p g j t -> tc xi p g j t")[0, 1])
        nc.sync.dma_start(wk_sb[:, 2], wk_re[:, 2])
        csv0 = load_csv(0)
        wq_sb = pw.tile([128, 3, NPG, 2, QD], e4m3, tag="wq")
        wq_re = wq3.ap().rearrange("ti g p j m -> p ti g j m")
        for ti in range(3):
            nc.sync.dma_start(wq_sb[:, ti], wq_re[:, ti])
        eye_sb = pw.tile([128, 128], f16, tag="eye")
        nc.sync.dma_start(eye_sb[:], eyeI.ap()[:])
        wv_sb = pw.tile([128, 3, NPG, 2, D], e4m3, tag="wv")
        nc.sync.dma_start(wv_sb[:], wv3.ap().rearrange("ti g p j m -> p ti g j m"))
        # masks/wp are declared here but loaded after the first p1 chunk's
        # DMAs are queued (they are first used by attn/outproj of chunk 0)
        masks_sb = pw.tile([128, NM, WIDE], f16, tag="masks")
        wp_sb = pw.tile([128, REP, C], f16, tag="wp")

        def load_late_consts():
            nc.sync.dma_start(
                masks_sb[:], masksI.ap().rearrange("(nd p) t -> p nd t", p=128))
            nc.sync.dma_start(
                wp_sb[:], wpT.ap().rearrange("(qc p) c -> p qc c", p=128))

        # round-robin evacuation helper (spread PSUM->SBUF copies)
        _evac_rr = [0]

        def evac(dst, src, engines=("act", "dve", "pool")):
            e = engines[_evac_rr[0] % len(engines)]
            _evac_rr[0] += 1
            if e == "act":
                nc.scalar.copy(dst, src)
            elif e == "dve":
                nc.vector.tensor_copy(dst, src)
            else:
                nc.gpsimd.tensor_copy(dst, src)

        # ================= phase 1: one t-chunk of projections =================
        # terms: (x index, w index): xh*wh + xh*wl_u + xl16*wh16
        TERMS = [(0, 0), (0, 1), (1, 2)]

        def p1_parts(tci, xt, csv, prefetch=None):
            """Return a list of thunks emitting one t-chunk of phase 1; they
            are interleaved between the previous chunk's attention heads."""
            t0 = tci * TC
            cs, sn, ve_t = csv
            state = {}

            def do_proj(kind, m):
                acc = pacc.tile([128, TC], f32, tag="acc", name="acc")
                w_sb = {"q": wq_sb, "k": wk_sb, "v": wv_sb}[kind]
                msl = slice(m * D, (m + 1) * D) if kind == "q" else slice(0, D)
                for i, (xi, wi) in enumerate(TERMS):
                    for g in range(NPG):
                        nc.tensor.matmul(
                            acc[:], w_sb[:, wi, g, :, msl], xt[:, xi, g, :, :],
                            start=(i == 0 and g == 0),
                            stop=(i == 2 and g == NPG - 1),
                            perf_mode=PM.DoubleRow)
                return acc

            def rope_emit(src_n, src_w, dst, toff):
                ta = ptmp.tile([128, TC], f16, tag="t", name="ta")
                nc.vector.tensor_mul(ta[:], src_n, cs[:])
                tb = ptmp.tile([128, TC], f16, tag="t", name="tb")
                nc.vector.tensor_mul(tb[:], src_w, sn[:])
                nc.vector.tensor_add(dst[:, toff:toff + TC], ta[:], tb[:])

            def do_qk(kind, m, acc):
                qn_all = state["qn"]
                qraw = ptmp.tile([128, TC], f16, tag="qr", name="qraw")
                nc.scalar.activation(qraw[:], acc[:], AF.Copy, scale=1.0 / _WS)
                sqr = ptmp.tile([128, TC], f16, tag="t", name="sqr")
                nc.vector.tensor_mul(sqr[:], qraw[:], qraw[:])
                if kind == "k":
                    # transposed sumsq -> rk as columns, folded into exp scale
                    rkT = psR.tile([128, 4], f32, tag="row", name="rkT")
                    for j in range(TC // 128):
                        nc.tensor.matmul(rkT[:, j:j + 1],
                                         sqr[:, j * 128:(j + 1) * 128],
                                         ones_sb[:], start=True, stop=True)
                    lncol = ptmp.tile([128, 4], f32, tag="rc", name="lncol")
                    nc.scalar.activation(lncol[:], rkT[:], AF.Ln,
                                         scale=1.0 / D, bias=eps_row[:, 0:1])
                    nc.scalar.activation(rk_cols[:, tci * 4:tci * 4 + 4],
                                         lncol[:], AF.Exp, scale=-0.5)
                    # k's swap + rope immediately: kT gates the next chunk's
                    # first scores matmul, so don't wait for the q streams
                    ksw = ptmp.tile([128, TC], f16, tag="t", name="ksw")
                    nc.sync.dma_start(ksw[0:64], qraw[64:128])
                    nc.sync.dma_start(ksw[64:128], qraw[0:64])
                    rope_emit(qraw[:], ksw[:], kT_sb, t0)
                else:
                    # stats deferred to part_rope so the Act queue is not
                    # blocked on the ss matmul, freeing the proj PSUM early
                    ss = psR.tile([1, TC], f32, tag="row", name="ss")
                    nc.tensor.matmul(ss[:], ones_sb[:], sqr[:],
                                     start=True, stop=True)
                    state[f"q{m}"] = (qraw, ss)

            def part_gate_k():
                if prefetch is not None:
                    prefetch()
                state["qn"] = pqn.tile([128, REP, TC], f16, tag="qn", name="qn")
                state["qsw"] = pqn.tile([128, REP, TC], f16, tag="qsw",
                                        name="qsw")
                zg = psR.tile([1, TC], f32, tag="row", name="zg")
                nc.tensor.matmul(zg[0:1, :], wg_sb[:],
                                 xt[0:VE_GATE_CH, 0, 0, 0, :],
                                 start=True, stop=True)
                ez = prow.tile([1, TC], f32, tag="g")
                nc.scalar.activation(ez[:], zg[:], AF.Exp, scale=-1.0 / 64.0)
                ez1 = prow.tile([1, TC], f32, tag="g")
                nc.vector.tensor_scalar_add(ez1[:], ez[:], 1.0)
                grow = prow.tile([1, TC], f16, tag="gh")
                nc.vector.reciprocal(grow[:], ez1[:])
                gbc = pbc.tile([128, TC], f16, tag="bc", name="gbc")
                nc.gpsimd.partition_broadcast(gbc[:], grow[:])
                state["gbc"] = gbc
                do_qk("k", 0, do_proj("k", 0))

            def part_q(m):
                def go():
                    do_qk("q", m, do_proj("q", m))
                return go

            def part_v():
                acc = do_proj("v", 0)
                vtmp = ptmp.tile([128, TC], f16, tag="t", name="vtmp")
                nc.vector.tensor_mul(vtmp[:], state["gbc"][:], ve_t[:])
                vfull = ptmp.tile([128, TC], f16, tag="t", name="vfull")
                nc.vector.scalar_tensor_tensor(
                    vfull[:], acc[:], 1.0 / _WS, vtmp[:],
                    mybir.AluOpType.mult, mybir.AluOpType.add)
                for j in range(TC // 128):
                    st = tci * (TC // 128) + j
                    vtr = psSY.tile([128, 128], f16, tag="sy", name="vtr")
                    nc.tensor.transpose(
                        vtr[:], vfull[:, j * 128:(j + 1) * 128], eye_sb[:])
                    evac(V_sb[:, st, :], vtr[:], ("act", "dve"))

            def stats_q(m):
                qn_all = state["qn"]
                qraw, ss = state[f"q{m}"]
                lnr = prow.tile([1, TC], f32, tag="r", name="lnr")
                nc.scalar.activation(lnr[:], ss[:], AF.Ln,
                                     scale=1.0 / D, bias=eps_row[0:1, :])
                rr = prow.tile([1, TC], f16, tag="rh", name="rr")
                nc.scalar.activation(rr[:], lnr[:], AF.Exp,
                                     scale=-0.5, bias=lncq_row[0:1, :])
                rbc = pbc.tile([128, TC], f16, tag="bc", name="rbc")
                nc.gpsimd.partition_broadcast(rbc[:], rr[:])
                nc.vector.tensor_mul(qn_all[:, m, :], rbc[:], qraw[:])

            def part_rope():
                qn_all, qsw_all = state["qn"], state["qsw"]
                nc.sync.dma_start(qsw_all[0:64], qn_all[64:128])
                nc.sync.dma_start(qsw_all[64:128], qn_all[0:64])
                for m in range(REP):
                    rope_emit(qn_all[:, m, :], qsw_all[:, m, :], qT_sb[m], t0)

            return [part_gate_k, part_q(0),
                    lambda: (part_q(1)(), stats_q(0)),
                    lambda: (part_q(2)(), stats_q(1)),
                    lambda: (part_v(), stats_q(2)),
                    part_rope]

        # ================= phase 2: attention + out-proj per chunk =============
        def attn_chunk(tci, fillers=()):
            t0 = tci * TC
            if win_finite:
                st_min = max(0, (t0 - window - 127) // 128 + 1)
            else:
                st_min = 0
            st_max = (t0 + TC - 1) // 128
            sts = list(range(st_min, st_max + 1))
            fillers = list(fillers)

            for h in range(REP):
                if fillers:
                    fillers.pop(0)()
                yU = psSY.tile([128, TC], f32, tag="sy", name="yU")
                den = pden.tile([128, TC], f16, tag="d", name="den")
                for idx, st in enumerate(sts):
                    s0 = st * 128
                    delta = t0 - s0
                    causal_p = delta <= 0
                    window_p = win_finite and delta > window - (TC - 1)
                    v0 = max(0, -delta) if causal_p else 0
                    v1 = min(TC, window - delta + 128) if window_p else TC
                    if idx == 0:
                        # full width so the yU start=True zeroes the whole
                        # bank; mask matmuls cover the out-of-range columns
                        # (the mask tiles encode the exact condition)
                        v0, v1 = 0, TC
                    nmm = int(causal_p) + int(window_p)
                    sc = psSY.tile([128, TC], f32, tag="sy", name="sc")
                    nc.tensor.matmul(sc[:, v0:v1], kT_sb[:, s0:s0 + 128],
                                     qT_sb[h][:, t0 + v0:t0 + v1],
                                     start=True, stop=(nmm == 0))
                    if causal_p:    # masked cols [v0, 128-delta)
                        c0, c1 = v0, min(v1, 128 - delta)
                        off = delta + (TC - 128)
                        nmm -= 1
                        nc.tensor.matmul(sc[:, c0:c1], eye_sb[:],
                                         masks_sb[:, 0, off + c0:off + c1],
                                         start=False, stop=(nmm == 0))
                    if window_p:    # masked cols suffix
                        c0 = max(v0, (window - delta + 1) // 128 * 128)
                        c1 = v1
                        off = delta - wmin
                        nmm -= 1
                        nc.tensor.matmul(sc[:, c0:c1], eye_sb[:],
                                         masks_sb[:, 1, off + c0:off + c1],
                                         start=False, stop=(nmm == 0))
                    P = pP.tile([128, TC], f16, tag="P", name="P")
                    nc.scalar.activation(P[:, v0:v1], sc[:, v0:v1], AF.Exp,
                                         scale=rk_cols[:, st:st + 1],
                                         bias=pshift_col[:, 0:1])
                    if idx == 0:    # full width: replaces a memset
                        nc.vector.tensor_copy(den[:], P[:])
                    else:
                        nc.vector.tensor_add(den[:, v0:v1], den[:, v0:v1],
                                             P[:, v0:v1])
                    nc.tensor.matmul(yU[:, v0:v1], V_sb[:, st, :],
                                     P[:, v0:v1], start=(idx == 0),
                                     stop=(idx == len(sts) - 1))
                drow = psR.tile([1, TC], f32, tag="row", name="dr")
                nc.tensor.matmul(drow[:], ones_sb[:], den[:],
                                 start=True, stop=True)
                dinv = prow.tile([1, TC], f16, tag="rh", name="dinv")
                nc.vector.reciprocal(dinv[:], drow[:])
                dbc = pbc.tile([128, TC], f16, tag="bc", name="dbc")
                nc.gpsimd.partition_broadcast(dbc[:], dinv[:])
                nc.vector.tensor_mul(yT_sb[h][:, t0:t0 + TC], dbc[:], yU[:])

            # ---- out-proj for this chunk ----
            # last chunk: finer DMA granularity to shrink the drain tail
            ngrp = 4 if tci == NTC - 1 else 2
            npc = NCC // ngrp
            for grp in range(ngrp):
                if fillers:
                    fillers.pop(0)()
                ot = pout.tile([128, npc, TC], f16, tag="ot", name="ot")
                for ci in range(npc):
                    cc = grp * npc + ci
                    o = psSY.tile([128, TC], f32, tag="sy", name="o")
                    for m in range(REP):
                        nc.tensor.matmul(
                            o[:], wp_sb[:, m, cc * 128:(cc + 1) * 128],
                            yT_sb[m][:, t0:t0 + TC],
                            start=(m == 0), stop=(m == REP - 1))
                    evac(ot[:, ci, :], o[:], ("act", "dve"))
                nc.sync.dma_start(
                    outT.ap().rearrange("(cc p) t -> p cc t", p=128)
                    [:, grp * npc:grp * npc + npc, t0:t0 + TC], ot[:])
            for f in fillers:
                f()

        # ================= interleaved schedule =================
        with nc.allow_low_precision(reason="fp16 validated vs numpy: 2e-3 max rel"):
            xt1 = load_x(1)
            csv1 = load_csv(1)
            xts = {0: xt0, 1: xt1}
            csvs = {0: csv0, 1: csv1}
            for p in p1_parts(0, xt0, csv0):
                p()
            load_late_consts()
            for tci in range(NTC):
                fillers = ()
                if tci + 1 < NTC:
                    prefetch = None
                    if tci + 2 < NTC:
                        def prefetch(t2=tci + 2):
                            xts[t2] = load_x(t2)
                            csvs[t2] = load_csv(t2)
                    fillers = p1_parts(tci + 1, xts[tci + 1], csvs[tci + 1],
                                       prefetch)
                attn_chunk(tci, fillers)

    nc.compile()
    nc._mask_cfg = {"wide": WIDE, "cmin": -(TC - 128), "wmin": wmin}
    return nc


def _qx_hilo(a):
    """x fp8 pair: a ~= hi + lo16/16."""
    import ml_dtypes
    hi = a.astype(ml_dtypes.float8_e4m3fn)
    hif = hi.astype(np.float32)
    lo16 = ((a - hif) * 16.0).astype(ml_dtypes.float8_e4m3fn)
    return hi, lo16


def _qw_tri(a):
    """weight fp8 triplet (prescaled): hi, lo_unscaled, hi/16."""
    import ml_dtypes
    hi = a.astype(ml_dtypes.float8_e4m3fn)
    hif = hi.astype(np.float32)
    lo_u = (a - hif).astype(ml_dtypes.float8_e4m3fn)
    hi16 = (hif / 16.0).astype(ml_dtypes.float8_e4m3fn)
    return hi, lo_u, hi16


def _pack_pairs(a3, outdim_last):
    """[3][rows(C), cols] -> [3, NPG, 128, 2, cols] pair-grouped layout."""
    out = np.stack([t.reshape(NPG, 2, 128, -1).transpose(0, 2, 1, 3)
                    for t in a3], axis=0)
    return np.ascontiguousarray(out)


def _prep_inputs(nc, window, x, ve, cos, sin, Wq, Wk, Wv, Wproj, Wg):
    import ml_dtypes
    f16t = ml_dtypes.float16 if hasattr(ml_dtypes, 'float16') else np.float16
    win_finite = 0 <= window < T
    cosT = np.ascontiguousarray(cos.reshape(T, D // 2).T)
    sinT = np.ascontiguousarray(sin.reshape(T, D // 2).T)
    cos2 = np.concatenate([cosT, cosT], axis=0).astype(np.float16)
    sin2m = np.concatenate([sinT, -sinT], axis=0).astype(np.float16)
    eye = np.eye(128, dtype=np.float16)
    ones = np.ones((128, 1), dtype=np.float16)

    ds = np.arange(128)[:, None]
    wcfg = nc._mask_cfg
    j = np.arange(wcfg["wide"])[None, :]
    mc = np.where(j + wcfg["cmin"] - ds >= 0, 0.0, _MASKVAL).astype(np.float16)
    rows = [mc]
    if win_finite:
        mw = np.where(j + wcfg["wmin"] - ds <= window, 0.0,
                      _MASKVAL).astype(np.float16)
        rows.append(mw)
    masks = np.concatenate(rows, axis=0)

    # x pairs per batch, chunk-major: [NTC, 2, 128, NPG, 2, TC]
    x3b = []
    for b in range(B):
        xT = x[b].T                                   # [C, T]
        tri = np.stack(_qx_hilo(xT), axis=0)          # [2, C, T]
        tri = tri.reshape(2, NPG, 2, 128, T).transpose(0, 3, 1, 2, 4)
        chunks = [tri[:, :, :, :, t0:t0 + TC] for t0 in range(0, T, TC)]
        x3b.append(np.ascontiguousarray(np.stack(chunks, axis=0)))

    in_maps = []
    for core in range(N_CORES):
        b, g = divmod(core, KV)
        sl_q = slice(g * QD, (g + 1) * QD)
        sl_d = slice(g * D, (g + 1) * D)
        wq_t = _pack_pairs(list(_qw_tri(Wq[sl_q].T * _WS)), QD)
        wk_t = _pack_pairs(list(_qw_tri(Wk[sl_d].T * _WS)), D)
        wv_t = _pack_pairs(list(_qw_tri(Wv[sl_d].T * _WS)), D)
        in_maps.append({
            "x3": x3b[b],
            "wq3": wq_t, "wk3": wk_t, "wv3": wv_t,
            "wpT": np.ascontiguousarray(Wproj[:, sl_q].T).astype(np.float16),
            "wg": np.ascontiguousarray(
                (Wg[g] * 64.0).reshape(VE_GATE_CH, 1)).astype(
                    __import__('ml_dtypes').float8_e4m3fn),
            "veT": np.ascontiguousarray(3.0 * ve[b, :, sl_d].T).astype(np.float16),
            "cos2": cos2, "sin2m": sin2m, "eyeI": eye, "onesI": ones,
            "masksI": masks,
        })
    return in_maps


def kernel(x, ve, cos, sin, Wq, Wk, Wv, Wproj, Wg, window, _trace=False):
    window = int(window)
    if window not in _CACHE:
        _CACHE[window] = _build(window)
    nc = _CACHE[window]

    in_maps = _prep_inputs(nc, window,
                           np.asarray(x, np.float32), np.asarray(ve, np.float32),
                           np.asarray(cos, np.float32), np.asarray(sin, np.float32),
                           np.asarray(Wq, np.float32), np.asarray(Wk, np.float32),
                           np.asarray(Wv, np.float32), np.asarray(Wproj, np.float32),
                           np.asarray(Wg, np.float32))

    res = run_bass_kernel_spmd(nc, in_maps, core_ids=list(range(N_CORES)),
                               trace=_trace)

    out = np.empty((B, T, C), dtype=np.float32)
    for b in range(B):
        acc = res.results[b * KV]["outT"].astype(np.float32)
        for g in range(1, KV):
            acc += res.results[b * KV + g]["outT"].astype(np.float32)
        out[b] = acc.T
    if _trace:
        kernel._last_trace = res
    return out


# revision 56
# speedup vs baseline: 1.4369x; 1.0175x over previous
"""Trainium2 Bass kernel for GQA causal sliding-window self-attention (v2).

Sharding: 8 cores = 2 (batch) x 4 (KV-head groups). Each core handles one
batch element and one KV head with its 3 GQA query heads. The output
projection is computed per-group against the matching Wproj column slice;
the 4 partial outputs per batch are summed on the host.

v2 speedups over the fp32r baseline:
- QKV projection in fp8e4 DoubleRow (2 k-tiles/pass, 0.5 cy/row) with a
  hi/lo error-compensated split: x*W ~= xh*wh + xh16*wl16 + xl16*wh16,
  where *16/*(1/16) scale folds keep every fp8 operand in range. All three
  terms accumulate in one fp32 PSUM at scale 32; the 1/32 comes out in the
  PSUM evacuation / rmsnorm constants. ~bf16-level accuracy at 0.75x the
  fp32r PE cost.
- Everything downstream in fp16 (scores, P, V, y, Wproj): same 1.0 cy/row
  PE rate as fp32r but 2x DVE throughput, half the DMA bytes, and no
  fast-path width constraint (allows exact [v0:v1] restriction).
- Softmax denominator accumulated on the (underloaded) DVE engine instead
  of ones-matmuls on the PE; only one final [1,TC] ones-matmul per
  (head, t-chunk).
- k-side rmsnorm scale folded into the exp activation's per-partition
  scale operand (P = exp(rk[s] * sc - 4)); the -4 shift keeps exp in fp16
  range. Column-constant shifts cancel in softmax.
- Phase interleaving: projection of t-chunk i+1 is issued before the
  attention of chunk i, so the PE never drains while Act/DVE catch up.
- Evacuations and broadcasts spread across Act/DVE/Pool by load.
"""

import os
import sys
import numpy as np

sys.path.insert(0, "/opt/trn_rl_repo")

from contextlib import ExitStack

from concourse import mybir, bacc, tile
from concourse.bass_utils import run_bass_kernel_spmd

f32 = mybir.dt.float32
f16 = mybir.dt.float16
e4m3 = mybir.dt.float8e4
AF = mybir.ActivationFunctionType
PM = mybir.MatmulPerfMode

B, T, C = 2, 2048, 1536
H, KV, D = 12, 4, 128
REP = H // KV          # 3 query heads per kv head
QD = REP * D           # 384
VE_GATE_CH = 12
N_CORES = 8
TC = 512               # t-chunk width
NTC = T // TC          # 4
NCC = C // 128         # 12 contraction chunks
NPG = NCC // 2         # 6 DoubleRow pair-groups
NST = T // 128         # 16 s-tiles

_EPS = float(np.finfo(np.float32).eps)
_WS = 32.0             # weight prescale for fp8 range
_LNCQ = float(np.log(1.2 * 1.2 / np.sqrt(D)))   # q-side fold: both 1.2s + 1/sqrt(D)
_MASKVAL = -30000.0    # fits fp16; * rk then exp -> exactly 0
_PSHIFT = -4.0         # logit shift so exp fits fp16 comfortably

_CACHE = {}


def _setup_act_tables():
    """Reorder activation-table sets so ln+exp(+square+copy) share one set.
    (Same trick as baseline; sim charges 1.28us per explicit table load.)"""
    try:
        import json
        import tempfile
        import concourse.hw_specs as hw_specs
        import concourse.bacc as bacc_mod
        from neuronxcc.driver.Job import Job
        from neuronxcc.driver.jobs.support.FindActInfo import findActInfoFile

        src = findActInfoFile(Job.getPackageDir(), "gen3")
        if not src or not os.path.exists(src):
            return
        src_dir = os.path.dirname(src)
        dst = os.path.join(tempfile.gettempdir(), "bass_act_pwp_lnexp")
        os.makedirs(dst, exist_ok=True)
        for f in os.listdir(src_dir):
            tgt = os.path.join(dst, f)
            if not os.path.exists(tgt):
                try:
                    os.symlink(os.path.join(src_dir, f), tgt)
                except OSError:
                    pass
        d = json.load(open(src))
        sets = d["act_func_sets"]
        idx = [i for i, s in enumerate(sets)
               if s["name"] == "natural_log_exp_and_others"]
        if not idx:
            return
        sets.insert(0, sets.pop(idx[0]))
        jp = os.path.join(dst, "act_info.json")
        if os.path.lexists(jp):
            os.remove(jp)
        json.dump(d, open(jp, "w"))
        os.environ["BASS_ACT_ROOT_JSON_PATH"] = jp

        orig = hw_specs.get_activation_tables

        def reordered(arch):
            t = orig(arch)
            key = "natural_log_exp_and_others"
            if key in t:
                out = {key: t[key]}
                out.update((k, v) for k, v in t.items() if k != key)
                return out
            return t

        hw_specs.get_activation_tables = reordered
        bacc_mod.get_activation_tables = reordered
    except Exception:
        pass


_setup_act_tables()


def _build(window: int):
    win_finite = 0 <= window < T
    # additive-mask tile classes, same scheme as baseline
    wdeltas = []
    if win_finite:
        dlt = window - (window % 128)
        while dlt + (TC - 1) > window:
            if dlt >= -(TC - 128):
                wdeltas.append(dlt)
            dlt -= 128
    wmin = min(wdeltas) if wdeltas else 0
    WIDE = TC + (TC - 128)
    NM = 2 if wdeltas else 1

    nc = bacc.Bacc("TRN2", target_bir_lowering=False, debug=False,
                   num_devices=N_CORES)

    # --- DRAM inputs ---
    # x in hi/lo16 fp8 pairs (xi: 0=hi, 1=lo16); weights in (hi, lo_u, hi16)
    # triplets. x*W ~= xh*wh + xh*wl_u + xl16*wh16, all at PSUM scale 32.
    # chunk-major so one (tci, xi) DMA is a contiguous [128, NPG*2*TC] copy
    x3 = nc.dram_tensor("x3", [NTC, 2, 128, NPG, 2, TC], e4m3,
                        kind="ExternalInput")
    wq3 = nc.dram_tensor("wq3", [3, NPG, 128, 2, QD], e4m3, kind="ExternalInput")
    wk3 = nc.dram_tensor("wk3", [3, NPG, 128, 2, D], e4m3, kind="ExternalInput")
    wv3 = nc.dram_tensor("wv3", [3, NPG, 128, 2, D], e4m3, kind="ExternalInput")
    wpT = nc.dram_tensor("wpT", [QD, C], f16, kind="ExternalInput")
    wg = nc.dram_tensor("wg", [VE_GATE_CH, 1], e4m3, kind="ExternalInput")
    veT = nc.dram_tensor("veT", [D, T], f16, kind="ExternalInput")
    cos2 = nc.dram_tensor("cos2", [128, T], f16, kind="ExternalInput")
    sin2m = nc.dram_tensor("sin2m", [128, T], f16, kind="ExternalInput")
    eyeI = nc.dram_tensor("eyeI", [128, 128], f16, kind="ExternalInput")
    onesI = nc.dram_tensor("onesI", [128, 1], f16, kind="ExternalInput")
    masksI = nc.dram_tensor("masksI", [NM * 128, WIDE], f16, kind="ExternalInput")
    outT = nc.dram_tensor("outT", [C, T], f16, kind="ExternalOutput")

    with tile.TileContext(nc) as tc, ExitStack() as ctx:
        # ---- persistent SBUF pools ----
        pw = ctx.enter_context(tc.tile_pool(name="pw", bufs=1))
        pbig = ctx.enter_context(tc.tile_pool(name="pbig", bufs=1))
        pxt = ctx.enter_context(tc.tile_pool(name="pxt", bufs=2))
        pcs = ctx.enter_context(tc.tile_pool(name="pcs", bufs=3))
        prow = ctx.enter_context(tc.tile_pool(name="prow", bufs=3))
        pbc = ctx.enter_context(tc.tile_pool(name="pbc", bufs=3))
        pqn = ctx.enter_context(tc.tile_pool(name="pqn", bufs=2))
        ptmp = ctx.enter_context(tc.tile_pool(name="ptmp", bufs=8))
        pP = ctx.enter_context(tc.tile_pool(name="pP", bufs=8))
        pden = ctx.enter_context(tc.tile_pool(name="pden", bufs=3))
        pout = ctx.enter_context(tc.tile_pool(name="pout", bufs=3))

        # ---- PSUM pools (8 banks) ----
        pacc = ctx.enter_context(tc.tile_pool(name="pacc", bufs=2, space="PSUM"))
        psR = ctx.enter_context(tc.tile_pool(name="psR", bufs=2, space="PSUM"))
        psSY = ctx.enter_context(tc.tile_pool(name="psSY", bufs=4, space="PSUM"))

        # ---- tiny constants first (DMA device is serial: order matters) ----
        wg_sb = pw.tile([VE_GATE_CH, 1], e4m3, tag="wg")
        nc.sync.dma_start(wg_sb[:], wg.ap()[:])
        ones_sb = pw.tile([128, 1], f16, tag="ones")
        nc.sync.dma_start(ones_sb[:], onesI.ap()[:])
        eps_row = pw.tile([128, 1], f32, tag="epsr")
        nc.vector.memset(eps_row[:], _EPS)
        lncq_row = pw.tile([128, 1], f32, tag="lncq")
        nc.vector.memset(lncq_row[:], _LNCQ)
        pshift_col = pw.tile([128, 1], f32, tag="pshift")
        nc.vector.memset(pshift_col[:], _PSHIFT)

        # ---- persistent activations ----
        qT_sb = [pbig.tile([128, T], f16, tag=f"qT{m}", name=f"qT{m}")
                 for m in range(REP)]
        kT_sb = pbig.tile([128, T], f16, tag="kT")
        V_sb = pbig.tile([128, NST, D], f16, tag="V")
        rk_cols = pbig.tile([128, NST], f32, tag="rkc")
        yT_sb = [pbig.tile([128, T], f16, tag=f"yT{m}", name=f"yT{m}")
                 for m in range(REP)]

        x_re = x3.ap().rearrange("tc xi p g j t -> tc xi p g j t")

        def load_x(tci):
            xt = pxt.tile([128, 2, NPG, 2, TC], e4m3, tag="xt", name=f"x{tci}")
            for xi in range(2):
                nc.sync.dma_start(xt[:, xi], x_re[tci, xi])
            return xt

        def load_csv(tci):
            t0 = tci * TC
            cs = pcs.tile([128, TC], f16, tag="cs")
            nc.sync.dma_start(cs[:], cos2.ap()[:, t0:t0 + TC])
            sn = pcs.tile([128, TC], f16, tag="sn")
            nc.sync.dma_start(sn[:], sin2m.ap()[:, t0:t0 + TC])
            ve_t = pcs.tile([128, TC], f16, tag="vet")
            nc.sync.dma_start(ve_t[:], veT.ap()[:, t0:t0 + TC])
            return cs, sn, ve_t

        # ---- PE p-state warm-up: keep the PE busy on junk matmuls while the
        # first x chunk streams in, so real matmuls start at full clock ----
        junk = pw.tile([128, TC], f16, tag="junk")
        nc.vector.memset(junk[:], 1.0)
        for _ in range(14):
            wu = psSY.tile([128, TC], f32, tag="sy", name="wu")
            nc.tensor.matmul(wu[:], junk[:, 0:128], junk[:],
                             start=True, stop=True)

        # ---- first x chunk + weights, in strict first-use order (the DMA
        # device is serial in the cost model, so issue order = arrival order)
        xt0 = pxt.tile([128, 2, NPG, 2, TC], e4m3, tag="xt", name="x0")
        nc.sync.dma_start(xt0[:, 0], x3.ap().rearrange(
            "tc xi p g j t -> tc xi p g j t")[0, 0])
        wk_sb = pw.tile([128, 3, NPG, 2, D], e4m3, tag="wk")
        wk_re = wk3.ap().rearrange("ti g p j m -> p ti g j m")
        nc.sync.dma_start(wk_sb[:, 0:2], wk_re[:, 0:2])
        nc.sync.dma_start(xt0[:, 1], x3.ap().rearrange(
            "tc xi p g j t -> tc xi p g j t")[0, 1])
        nc.sync.dma_start(wk_sb[:, 2], wk_re[:, 2])
        csv0 = load_csv(0)
        wq_sb = pw.tile([128, 3, NPG, 2, QD], e4m3, tag="wq")
        wq_re = wq3.ap().rearrange("ti g p j m -> p ti g j m")
        for ti in range(3):
            nc.sync.dma_start(wq_sb[:, ti], wq_re[:, ti])
        eye_sb = pw.tile([128, 128], f16, tag="eye")
        nc.sync.dma_start(eye_sb[:], eyeI.ap()[:])
        wv_sb = pw.tile([128, 3, NPG, 2, D], e4m3, tag="wv")
        nc.sync.dma_start(wv_sb[:], wv3.ap().rearrange("ti g p j m -> p ti g j m"))
        # masks/wp are declared here but loaded after the first p1 chunk's
        # DMAs are queued (they are first used by attn/outproj of chunk 0)
        masks_sb = pw.tile([128, NM, WIDE], f16, tag="masks")
        wp_sb = pw.tile([128, REP, C], f16, tag="wp")

        def load_late_consts():
            nc.sync.dma_start(
                masks_sb[:], masksI.ap().rearrange("(nd p) t -> p nd t", p=128))
            nc.sync.dma_start(
                wp_sb[:], wpT.ap().rearrange("(qc p) c -> p qc c", p=128))

        # round-robin evacuation helper (spread PSUM->SBUF copies)
        _evac_rr = [0]

        def evac(dst, src, engines=("act", "dve", "pool")):
            e = engines[_evac_rr[0] % len(engines)]
            _evac_rr[0] += 1
            if e == "act":
                nc.scalar.copy(dst, src)
            elif e == "dve":
                nc.vector.tensor_copy(dst, src)
            else:
                nc.gpsimd.tensor_copy(dst, src)

        # ================= phase 1: one t-chunk of projections =================
        # terms: (x index, w index): xh*wh + xh*wl_u + xl16*wh16
        TERMS = [(0, 0), (0, 1), (1, 2)]

        def p1_parts(tci, xt, csv, prefetch=None):
            """Return a list of thunks emitting one t-chunk of phase 1; they
            are interleaved between the previous chunk's attention heads."""
            t0 = tci * TC
            cs, sn, ve_t = csv
            state = {}

            def do_proj(kind, m):
                acc = pacc.tile([128, TC], f32, tag="acc", name="acc")
                w_sb = {"q": wq_sb, "k": wk_sb, "v": wv_sb}[kind]
                msl = slice(m * D, (m + 1) * D) if kind == "q" else slice(0, D)
                for i, (xi, wi) in enumerate(TERMS):
                    for g in range(NPG):
                        nc.tensor.matmul(
                            acc[:], w_sb[:, wi, g, :, msl], xt[:, xi, g, :, :],
                            start=(i == 0 and g == 0),
                            stop=(i == 2 and g == NPG - 1),
                            perf_mode=PM.DoubleRow)
                return acc

            def rope_emit(src_n, src_w, dst, toff):
                ta = ptmp.tile([128, TC], f16, tag="t", name="ta")
                nc.vector.tensor_mul(ta[:], src_n, cs[:])
                tb = ptmp.tile([128, TC], f16, tag="t", name="tb")
                nc.vector.tensor_mul(tb[:], src_w, sn[:])
                nc.vector.tensor_add(dst[:, toff:toff + TC], ta[:], tb[:])

            def do_qk(kind, m, acc):
                qn_all = state["qn"]
                qraw = ptmp.tile([128, TC], f16, tag="qr", name="qraw")
                nc.scalar.activation(qraw[:], acc[:], AF.Copy, scale=1.0 / _WS)
                sqr = ptmp.tile([128, TC], f16, tag="t", name="sqr")
                nc.vector.tensor_mul(sqr[:], qraw[:], qraw[:])
                if kind == "k":
                    # transposed sumsq -> rk as columns, folded into exp scale
                    rkT = psR.tile([128, 4], f32, tag="row", name="rkT")
                    for j in range(TC // 128):
                        nc.tensor.matmul(rkT[:, j:j + 1],
                                         sqr[:, j * 128:(j + 1) * 128],
                                         ones_sb[:], start=True, stop=True)
                    lncol = ptmp.tile([128, 4], f32, tag="rc", name="lncol")
                    nc.scalar.activation(lncol[:], rkT[:], AF.Ln,
                                         scale=1.0 / D, bias=eps_row[:, 0:1])
                    nc.scalar.activation(rk_cols[:, tci * 4:tci * 4 + 4],
                                         lncol[:], AF.Exp, scale=-0.5)
                    # k's swap + rope immediately: kT gates the next chunk's
                    # first scores matmul, so don't wait for the q streams
                    ksw = ptmp.tile([128, TC], f16, tag="t", name="ksw")
                    nc.sync.dma_start(ksw[0:64], qraw[64:128])
                    nc.sync.dma_start(ksw[64:128], qraw[0:64])
                    rope_emit(qraw[:], ksw[:], kT_sb, t0)
                else:
                    # stats deferred so the Act queue is not blocked on the
                    # ss matmul, freeing the proj PSUM early
                    ss = psR.tile([1, TC], f32, tag="row", name="ss")
                    nc.tensor.matmul(ss[:], ones_sb[:], sqr[:],
                                     start=True, stop=True)
                    state[f"q{m}"] = (qraw, ss)

            def part_gate_k():
                if prefetch is not None:
                    prefetch()
                state["qn"] = pqn.tile([128, REP, TC], f16, tag="qn", name="qn")
                state["qsw"] = pqn.tile([128, REP, TC], f16, tag="qsw",
                                        name="qsw")
                zg = psR.tile([1, TC], f32, tag="row", name="zg")
                nc.tensor.matmul(zg[0:1, :], wg_sb[:],
                                 xt[0:VE_GATE_CH, 0, 0, 0, :],
                                 start=True, stop=True)
                ez = prow.tile([1, TC], f32, tag="g")
                nc.scalar.activation(ez[:], zg[:], AF.Exp, scale=-1.0 / 64.0)
                ez1 = prow.tile([1, TC], f32, tag="g")
                nc.vector.tensor_scalar_add(ez1[:], ez[:], 1.0)
                grow = prow.tile([1, TC], f16, tag="gh")
                nc.vector.reciprocal(grow[:], ez1[:])
                gbc = pbc.tile([128, TC], f16, tag="bc", name="gbc")
                nc.gpsimd.partition_broadcast(gbc[:], grow[:])
                state["gbc"] = gbc
                do_qk("k", 0, do_proj("k", 0))

            def part_q(m):
                def go():
                    do_qk("q", m, do_proj("q", m))
                return go

            def part_v():
                acc = do_proj("v", 0)
                vtmp = ptmp.tile([128, TC], f16, tag="t", name="vtmp")
                nc.vector.tensor_mul(vtmp[:], state["gbc"][:], ve_t[:])
                vfull = ptmp.tile([128, TC], f16, tag="t", name="vfull")
                nc.vector.scalar_tensor_tensor(
                    vfull[:], acc[:], 1.0 / _WS, vtmp[:],
                    mybir.AluOpType.mult, mybir.AluOpType.add)
                for j in range(TC // 128):
                    st = tci * (TC // 128) + j
                    vtr = psSY.tile([128, 128], f16, tag="sy", name="vtr")
                    nc.tensor.transpose(
                        vtr[:], vfull[:, j * 128:(j + 1) * 128], eye_sb[:])
                    evac(V_sb[:, st, :], vtr[:], ("dve", "act"))

            def stats_q(m):
                qn_all = state["qn"]
                qraw, ss = state[f"q{m}"]
                lnr = prow.tile([1, TC], f32, tag="r", name="lnr")
                nc.scalar.activation(lnr[:], ss[:], AF.Ln,
                                     scale=1.0 / D, bias=eps_row[0:1, :])
                rr = prow.tile([1, TC], f16, tag="rh", name="rr")
                nc.scalar.activation(rr[:], lnr[:], AF.Exp,
                                     scale=-0.5, bias=lncq_row[0:1, :])
                rbc = pbc.tile([128, TC], f16, tag="bc", name="rbc")
                nc.gpsimd.partition_broadcast(rbc[:], rr[:])
                nc.vector.tensor_mul(qn_all[:, m, :], rbc[:], qraw[:])

            def part_rope():
                qn_all, qsw_all = state["qn"], state["qsw"]
                nc.sync.dma_start(qsw_all[0:64], qn_all[64:128])
                nc.sync.dma_start(qsw_all[64:128], qn_all[0:64])
                for m in range(REP):
                    rope_emit(qn_all[:, m, :], qsw_all[:, m, :], qT_sb[m], t0)

            return [part_gate_k, part_q(0),
                    lambda: (part_q(1)(), stats_q(0)),
                    lambda: (part_q(2)(), stats_q(1)),
                    lambda: (part_v(), stats_q(2)),
                    part_rope]

        # ================= phase 2: attention + out-proj per chunk =============
        def attn_chunk(tci, fillers=()):
            t0 = tci * TC
            if win_finite:
                st_min = max(0, (t0 - window - 127) // 128 + 1)
            else:
                st_min = 0
            st_max = (t0 + TC - 1) // 128
            sts = list(range(st_min, st_max + 1))
            # put a fully-valid tile first when one exists: its full-width
            # start=True zeroes the yU bank without widening a masked tile
            def full_valid(st):
                delta = t0 - st * 128
                return (delta >= 128 and
                        (not win_finite or delta + TC - 1 <= window))
            fv = [st for st in sts if full_valid(st)]
            for i, st in enumerate(fv[:2]):
                sts.remove(st)
                sts.insert(i, st)
            nden = 2 if len(fv) >= 2 else 1
            fillers = list(fillers)

            for h in range(REP):
                if fillers:
                    fillers.pop(0)()
                yU = psSY.tile([128, TC], f32, tag="sy", name="yU")
                dens = [pden.tile([128, TC], f16, tag="d", name="den")]
                if nden == 2:
                    den2 = pden.tile([128, TC], f16, tag="d2", name="den2")
                    dens.append(den2)
                for idx, st in enumerate(sts):
                    s0 = st * 128
                    delta = t0 - s0
                    causal_p = delta <= 0
                    window_p = win_finite and delta > window - (TC - 1)
                    v0 = max(0, -delta) if causal_p else 0
                    v1 = min(TC, window - delta + 128) if window_p else TC
                    if idx == 0:
                        # full width so the yU start=True zeroes the whole
                        # bank; mask matmuls cover the out-of-range columns
                        # (the mask tiles encode the exact condition)
                        v0, v1 = 0, TC
                    nmm = int(causal_p) + int(window_p)
                    sc = psSY.tile([128, TC], f32, tag="sy", name="sc")
                    nc.tensor.matmul(sc[:, v0:v1], kT_sb[:, s0:s0 + 128],
                                     qT_sb[h][:, t0 + v0:t0 + v1],
                                     start=True, stop=(nmm == 0))
                    if causal_p:    # masked cols [v0, 128-delta)
                        c0, c1 = v0, min(v1, 128 - delta)
                        off = delta + (TC - 128)
                        nmm -= 1
                        nc.tensor.matmul(sc[:, c0:c1], eye_sb[:],
                                         masks_sb[:, 0, off + c0:off + c1],
                                         start=False, stop=(nmm == 0))
                    if window_p:    # masked cols suffix
                        c0 = max(v0, (window - delta + 1) // 128 * 128)
                        c1 = v1
                        off = delta - wmin
                        nmm -= 1
                        nc.tensor.matmul(sc[:, c0:c1], eye_sb[:],
                                         masks_sb[:, 1, off + c0:off + c1],
                                         start=False, stop=(nmm == 0))
                    P = pP.tile([128, TC], f16, tag="P", name="P")
                    nc.scalar.activation(P[:, v0:v1], sc[:, v0:v1], AF.Exp,
                                         scale=rk_cols[:, st:st + 1],
                                         bias=pshift_col[:, 0:1])
                    den = dens[idx % nden]
                    if idx < nden:  # full width: replaces a memset
                        nc.vector.tensor_copy(den[:], P[:])
                    else:
                        nc.vector.tensor_add(den[:, v0:v1], den[:, v0:v1],
                                             P[:, v0:v1])
                    nc.tensor.matmul(yU[:, v0:v1], V_sb[:, st, :],
                                     P[:, v0:v1], start=(idx == 0),
                                     stop=(idx == len(sts) - 1))
                drow = psR.tile([1, TC], f32, tag="row", name="dr")
                for di in range(nden):
                    nc.tensor.matmul(drow[:], ones_sb[:], dens[di][:],
                                     start=(di == 0), stop=(di == nden - 1))
                dinv = prow.tile([1, TC], f16, tag="rh", name="dinv")
                nc.vector.reciprocal(dinv[:], drow[:])
                dbc = pbc.tile([128, TC], f16, tag="bc", name="dbc")
                nc.gpsimd.partition_broadcast(dbc[:], dinv[:])
                nc.vector.tensor_mul(yT_sb[h][:, t0:t0 + TC], dbc[:], yU[:])

            # ---- out-proj for this chunk ----
            # last chunk: finer DMA granularity to shrink the drain tail
            ngrp = 4 if tci == NTC - 1 else 2
            npc = NCC // ngrp
            for grp in range(ngrp):
                if fillers:
                    fillers.pop(0)()
                ot = pout.tile([128, npc, TC], f16, tag="ot", name="ot")
                for ci in range(npc):
                    cc = grp * npc + ci
                    o = psSY.tile([128, TC], f32, tag="sy", name="o")
                    for m in range(REP):
                        nc.tensor.matmul(
                            o[:], wp_sb[:, m, cc * 128:(cc + 1) * 128],
                            yT_sb[m][:, t0:t0 + TC],
                            start=(m == 0), stop=(m == REP - 1))
                    evac(ot[:, ci, :], o[:], ("act", "dve"))
                nc.sync.dma_start(
                    outT.ap().rearrange("(cc p) t -> p cc t", p=128)
                    [:, grp * npc:grp * npc + npc, t0:t0 + TC], ot[:])
            for f in fillers:
                f()

        # ================= interleaved schedule =================
        with nc.allow_low_precision(reason="fp16 validated vs numpy: 2e-3 max rel"):
            xt1 = load_x(1)
            csv1 = load_csv(1)
            xts = {0: xt0, 1: xt1}
            csvs = {0: csv0, 1: csv1}
            for p in p1_parts(0, xt0, csv0):
                p()
            load_late_consts()
            for tci in range(NTC):
                fillers = ()
                if tci + 1 < NTC:
                    prefetch = None
                    if tci + 2 < NTC:
                        def prefetch(t2=tci + 2):
                            xts[t2] = load_x(t2)
                            csvs[t2] = load_csv(t2)
                    fillers = p1_parts(tci + 1, xts[tci + 1], csvs[tci + 1],
                                       prefetch)
                attn_chunk(tci, fillers)

    nc.compile()
    nc._mask_cfg = {"wide": WIDE, "cmin": -(TC - 128), "wmin": wmin}
    return nc


def _qx_hilo(a):
    """x fp8 pair: a ~= hi + lo16/16."""
    import ml_dtypes
    hi = a.astype(ml_dtypes.float8_e4m3fn)
    hif = hi.astype(np.float32)
    lo16 = ((a - hif) * 16.0).astype(ml_dtypes.float8_e4m3fn)
    return hi, lo16


def _qw_tri(a):
    """weight fp8 triplet (prescaled): hi, lo_unscaled, hi/16."""
    import ml_dtypes
    hi = a.astype(ml_dtypes.float8_e4m3fn)
    hif = hi.astype(np.float32)
    lo_u = (a - hif).astype(ml_dtypes.float8_e4m3fn)
    hi16 = (hif / 16.0).astype(ml_dtypes.float8_e4m3fn)
    return hi, lo_u, hi16


def _pack_pairs(a3, outdim_last):
    """[3][rows(C), cols] -> [3, NPG, 128, 2, cols] pair-grouped layout."""
    out = np.stack([t.reshape(NPG, 2, 128, -1).transpose(0, 2, 1, 3)
                    for t in a3], axis=0)
    return np.ascontiguousarray(out)


def _prep_inputs(nc, window, x, ve, cos, sin, Wq, Wk, Wv, Wproj, Wg):
    import ml_dtypes
    f16t = ml_dtypes.float16 if hasattr(ml_dtypes, 'float16') else np.float16
    win_finite = 0 <= window < T
    cosT = np.ascontiguousarray(cos.reshape(T, D // 2).T)
    sinT = np.ascontiguousarray(sin.reshape(T, D // 2).T)
    cos2 = np.concatenate([cosT, cosT], axis=0).astype(np.float16)
    sin2m = np.concatenate([sinT, -sinT], axis=0).astype(np.float16)
    eye = np.eye(128, dtype=np.float16)
    ones = np.ones((128, 1), dtype=np.float16)

    ds = np.arange(128)[:, None]
    wcfg = nc._mask_cfg
    j = np.arange(wcfg["wide"])[None, :]
    mc = np.where(j + wcfg["cmin"] - ds >= 0, 0.0, _MASKVAL).astype(np.float16)
    rows = [mc]
    if win_finite:
        mw = np.where(j + wcfg["wmin"] - ds <= window, 0.0,
                      _MASKVAL).astype(np.float16)
        rows.append(mw)
    masks = np.concatenate(rows, axis=0)

    # x pairs per batch, chunk-major: [NTC, 2, 128, NPG, 2, TC]
    x3b = []
    for b in range(B):
        xT = x[b].T                                   # [C, T]
        tri = np.stack(_qx_hilo(xT), axis=0)          # [2, C, T]
        tri = tri.reshape(2, NPG, 2, 128, T).transpose(0, 3, 1, 2, 4)
        chunks = [tri[:, :, :, :, t0:t0 + TC] for t0 in range(0, T, TC)]
        x3b.append(np.ascontiguousarray(np.stack(chunks, axis=0)))

    in_maps = []
    for core in range(N_CORES):
        b, g = divmod(core, KV)
        sl_q = slice(g * QD, (g + 1) * QD)
        sl_d = slice(g * D, (g + 1) * D)
        wq_t = _pack_pairs(list(_qw_tri(Wq[sl_q].T * _WS)), QD)
        wk_t = _pack_pairs(list(_qw_tri(Wk[sl_d].T * _WS)), D)
        wv_t = _pack_pairs(list(_qw_tri(Wv[sl_d].T * _WS)), D)
        in_maps.append({
            "x3": x3b[b],
            "wq3": wq_t, "wk3": wk_t, "wv3": wv_t,
            "wpT": np.ascontiguousarray(Wproj[:, sl_q].T).astype(np.float16),
            "wg": np.ascontiguousarray(
                (Wg[g] * 64.0).reshape(VE_GATE_CH, 1)).astype(
                    __import__('ml_dtypes').float8_e4m3fn),
            "veT": np.ascontiguousarray(3.0 * ve[b, :, sl_d].T).astype(np.float16),
            "cos2": cos2, "sin2m": sin2m, "eyeI": eye, "onesI": ones,
            "masksI": masks,
        })
    return in_maps


def kernel(x, ve, cos, sin, Wq, Wk, Wv, Wproj, Wg, window, _trace=False):
    window = int(window)
    if window not in _CACHE:
        _CACHE[window] = _build(window)
    nc = _CACHE[window]

    in_maps = _prep_inputs(nc, window,
                           np.asarray(x, np.float32), np.asarray(ve, np.float32),
                           np.asarray(cos, np.float32), np.asarray(sin, np.float32),
                           np.asarray(Wq, np.float32), np.asarray(Wk, np.float32),
                           np.asarray(Wv, np.float32), np.asarray(Wproj, np.float32),
                           np.asarray(Wg, np.float32))

    res = run_bass_kernel_spmd(nc, in_maps, core_ids=list(range(N_CORES)),
                               trace=_trace)

    out = np.empty((B, T, C), dtype=np.float32)
    for b in range(B):
        acc = res.results[b * KV]["outT"].astype(np.float32)
        for g in range(1, KV):
            acc += res.results[b * KV + g]["outT"].astype(np.float32)
        out[b] = acc.T
    if _trace:
        kernel._last_trace = res
    return out


# revision 57
# speedup vs baseline: 1.4378x; 1.0007x over previous
"""Trainium2 Bass kernel for GQA causal sliding-window self-attention (v2).

Sharding: 8 cores = 2 (batch) x 4 (KV-head groups). Each core handles one
batch element and one KV head with its 3 GQA query heads. The output
projection is computed per-group against the matching Wproj column slice;
the 4 partial outputs per batch are summed on the host.

v2 speedups over the fp32r baseline:
- QKV projection in fp8e4 DoubleRow (2 k-tiles/pass, 0.5 cy/row) with a
  hi/lo error-compensated split: x*W ~= xh*wh + xh16*wl16 + xl16*wh16,
  where *16/*(1/16) scale folds keep every fp8 operand in range. All three
  terms accumulate in one fp32 PSUM at scale 32; the 1/32 comes out in the
  PSUM evacuation / rmsnorm constants. ~bf16-level accuracy at 0.75x the
  fp32r PE cost.
- Everything downstream in fp16 (scores, P, V, y, Wproj): same 1.0 cy/row
  PE rate as fp32r but 2x DVE throughput, half the DMA bytes, and no
  fast-path width constraint (allows exact [v0:v1] restriction).
- Softmax denominator accumulated on the (underloaded) DVE engine instead
  of ones-matmuls on the PE; only one final [1,TC] ones-matmul per
  (head, t-chunk).
- k-side rmsnorm scale folded into the exp activation's per-partition
  scale operand (P = exp(rk[s] * sc - 4)); the -4 shift keeps exp in fp16
  range. Column-constant shifts cancel in softmax.
- Phase interleaving: projection of t-chunk i+1 is issued before the
  attention of chunk i, so the PE never drains while Act/DVE catch up.
- Evacuations and broadcasts spread across Act/DVE/Pool by load.
"""

import os
import sys
import numpy as np

sys.path.insert(0, "/opt/trn_rl_repo")

from contextlib import ExitStack

from concourse import mybir, bacc, tile
from concourse.bass_utils import run_bass_kernel_spmd

f32 = mybir.dt.float32
f16 = mybir.dt.float16
e4m3 = mybir.dt.float8e4
AF = mybir.ActivationFunctionType
PM = mybir.MatmulPerfMode

B, T, C = 2, 2048, 1536
H, KV, D = 12, 4, 128
REP = H // KV          # 3 query heads per kv head
QD = REP * D           # 384
VE_GATE_CH = 12
N_CORES = 8
TC = 512               # t-chunk width
NTC = T // TC          # 4
NCC = C // 128         # 12 contraction chunks
NPG = NCC // 2         # 6 DoubleRow pair-groups
NST = T // 128         # 16 s-tiles

_EPS = float(np.finfo(np.float32).eps)
_WS = 32.0             # weight prescale for fp8 range
_LNCQ = float(np.log(1.2 * 1.2 / np.sqrt(D)))   # q-side fold: both 1.2s + 1/sqrt(D)
_MASKVAL = -30000.0    # fits fp16; * rk then exp -> exactly 0
_PSHIFT = -4.0         # logit shift so exp fits fp16 comfortably

_CACHE = {}


def _setup_act_tables():
    """Reorder activation-table sets so ln+exp(+square+copy) share one set.
    (Same trick as baseline; sim charges 1.28us per explicit table load.)"""
    try:
        import json
        import tempfile
        import concourse.hw_specs as hw_specs
        import concourse.bacc as bacc_mod
        from neuronxcc.driver.Job import Job
        from neuronxcc.driver.jobs.support.FindActInfo import findActInfoFile

        src = findActInfoFile(Job.getPackageDir(), "gen3")
        if not src or not os.path.exists(src):
            return
        src_dir = os.path.dirname(src)
        dst = os.path.join(tempfile.gettempdir(), "bass_act_pwp_lnexp")
        os.makedirs(dst, exist_ok=True)
        for f in os.listdir(src_dir):
            tgt = os.path.join(dst, f)
            if not os.path.exists(tgt):
                try:
                    os.symlink(os.path.join(src_dir, f), tgt)
                except OSError:
                    pass
        d = json.load(open(src))
        sets = d["act_func_sets"]
        idx = [i for i, s in enumerate(sets)
               if s["name"] == "natural_log_exp_and_others"]
        if not idx:
            return
        sets.insert(0, sets.pop(idx[0]))
        jp = os.path.join(dst, "act_info.json")
        if os.path.lexists(jp):
            os.remove(jp)
        json.dump(d, open(jp, "w"))
        os.environ["BASS_ACT_ROOT_JSON_PATH"] = jp

        orig = hw_specs.get_activation_tables

        def reordered(arch):
            t = orig(arch)
            key = "natural_log_exp_and_others"
            if key in t:
                out = {key: t[key]}
                out.update((k, v) for k, v in t.items() if k != key)
                return out
            return t

        hw_specs.get_activation_tables = reordered
        bacc_mod.get_activation_tables = reordered
    except Exception:
        pass


_setup_act_tables()


def _build(window: int):
    win_finite = 0 <= window < T
    # additive-mask tile classes, same scheme as baseline
    wdeltas = []
    if win_finite:
        dlt = window - (window % 128)
        while dlt + (TC - 1) > window:
            if dlt >= -(TC - 128):
                wdeltas.append(dlt)
            dlt -= 128
    wmin = min(wdeltas) if wdeltas else 0
    WIDE = TC + (TC - 128)
    NM = 2 if wdeltas else 1

    nc = bacc.Bacc("TRN2", target_bir_lowering=False, debug=False,
                   num_devices=N_CORES)

    # --- DRAM inputs ---
    # x in hi/lo16 fp8 pairs (xi: 0=hi, 1=lo16); weights in (hi, lo_u, hi16)
    # triplets. x*W ~= xh*wh + xh*wl_u + xl16*wh16, all at PSUM scale 32.
    # chunk-major so one (tci, xi) DMA is a contiguous [128, NPG*2*TC] copy
    x3 = nc.dram_tensor("x3", [NTC, 2, 128, NPG, 2, TC], e4m3,
                        kind="ExternalInput")
    wq3 = nc.dram_tensor("wq3", [3, NPG, 128, 2, QD], e4m3, kind="ExternalInput")
    wk3 = nc.dram_tensor("wk3", [3, NPG, 128, 2, D], e4m3, kind="ExternalInput")
    wv3 = nc.dram_tensor("wv3", [3, NPG, 128, 2, D], e4m3, kind="ExternalInput")
    wpT = nc.dram_tensor("wpT", [QD, C], f16, kind="ExternalInput")
    wg = nc.dram_tensor("wg", [VE_GATE_CH, 1], e4m3, kind="ExternalInput")
    veT = nc.dram_tensor("veT", [D, T], f16, kind="ExternalInput")
    cos2 = nc.dram_tensor("cos2", [128, T], f16, kind="ExternalInput")
    sin2m = nc.dram_tensor("sin2m", [128, T], f16, kind="ExternalInput")
    eyeI = nc.dram_tensor("eyeI", [128, 128], f16, kind="ExternalInput")
    onesI = nc.dram_tensor("onesI", [128, 1], f16, kind="ExternalInput")
    masksI = nc.dram_tensor("masksI", [NM * 128, WIDE], f16, kind="ExternalInput")
    outT = nc.dram_tensor("outT", [C, T], f16, kind="ExternalOutput")

    with tile.TileContext(nc) as tc, ExitStack() as ctx:
        # ---- persistent SBUF pools ----
        pw = ctx.enter_context(tc.tile_pool(name="pw", bufs=1))
        pbig = ctx.enter_context(tc.tile_pool(name="pbig", bufs=1))
        pxt = ctx.enter_context(tc.tile_pool(name="pxt", bufs=2))
        pcs = ctx.enter_context(tc.tile_pool(name="pcs", bufs=3))
        prow = ctx.enter_context(tc.tile_pool(name="prow", bufs=3))
        pbc = ctx.enter_context(tc.tile_pool(name="pbc", bufs=3))
        pqn = ctx.enter_context(tc.tile_pool(name="pqn", bufs=2))
        ptmp = ctx.enter_context(tc.tile_pool(name="ptmp", bufs=8))
        pP = ctx.enter_context(tc.tile_pool(name="pP", bufs=8))
        pden = ctx.enter_context(tc.tile_pool(name="pden", bufs=3))
        pout = ctx.enter_context(tc.tile_pool(name="pout", bufs=3))

        # ---- PSUM pools (8 banks) ----
        pacc = ctx.enter_context(tc.tile_pool(name="pacc", bufs=2, space="PSUM"))
        psR = ctx.enter_context(tc.tile_pool(name="psR", bufs=2, space="PSUM"))
        psSY = ctx.enter_context(tc.tile_pool(name="psSY", bufs=4, space="PSUM"))

        # ---- tiny constants first (DMA device is serial: order matters) ----
        wg_sb = pw.tile([VE_GATE_CH, 1], e4m3, tag="wg")
        nc.sync.dma_start(wg_sb[:], wg.ap()[:])
        ones_sb = pw.tile([128, 1], f16, tag="ones")
        nc.sync.dma_start(ones_sb[:], onesI.ap()[:])
        eps_row = pw.tile([128, 1], f32, tag="epsr")
        nc.vector.memset(eps_row[:], _EPS)
        lncq_row = pw.tile([128, 1], f32, tag="lncq")
        nc.vector.memset(lncq_row[:], _LNCQ)
        pshift_col = pw.tile([128, 1], f32, tag="pshift")
        nc.vector.memset(pshift_col[:], _PSHIFT)

        # ---- persistent activations ----
        qT_sb = [pbig.tile([128, T], f16, tag=f"qT{m}", name=f"qT{m}")
                 for m in range(REP)]
        kT_sb = pbig.tile([128, T], f16, tag="kT")
        V_sb = pbig.tile([128, NST, D], f16, tag="V")
        rk_cols = pbig.tile([128, NST], f32, tag="rkc")
        yT_sb = [pbig.tile([128, T], f16, tag=f"yT{m}", name=f"yT{m}")
                 for m in range(REP)]

        x_re = x3.ap().rearrange("tc xi p g j t -> tc xi p g j t")

        def load_x(tci):
            xt = pxt.tile([128, 2, NPG, 2, TC], e4m3, tag="xt", name=f"x{tci}")
            for xi in range(2):
                nc.sync.dma_start(xt[:, xi], x_re[tci, xi])
            return xt

        def load_csv(tci):
            t0 = tci * TC
            cs = pcs.tile([128, TC], f16, tag="cs")
            nc.sync.dma_start(cs[:], cos2.ap()[:, t0:t0 + TC])
            sn = pcs.tile([128, TC], f16, tag="sn")
            nc.sync.dma_start(sn[:], sin2m.ap()[:, t0:t0 + TC])
            ve_t = pcs.tile([128, TC], f16, tag="vet")
            nc.sync.dma_start(ve_t[:], veT.ap()[:, t0:t0 + TC])
            return cs, sn, ve_t

        # ---- PE p-state warm-up: keep the PE busy on junk matmuls while the
        # first x chunk streams in, so real matmuls start at full clock ----
        junk = pw.tile([128, TC], f16, tag="junk")
        nc.vector.memset(junk[:], 1.0)
        for _ in range(14):
            wu = psSY.tile([128, TC], f32, tag="sy", name="wu")
            nc.tensor.matmul(wu[:], junk[:, 0:128], junk[:],
                             start=True, stop=True)

        # ---- first x chunk + weights, in strict first-use order (the DMA
        # device is serial in the cost model, so issue order = arrival order)
        xt0 = pxt.tile([128, 2, NPG, 2, TC], e4m3, tag="xt", name="x0")
        nc.sync.dma_start(xt0[:, 0], x3.ap().rearrange(
            "tc xi p g j t -> tc xi p g j t")[0, 0])
        wk_sb = pw.tile([128, 3, NPG, 2, D], e4m3, tag="wk")
        wk_re = wk3.ap().rearrange("ti g p j m -> p ti g j m")
        nc.sync.dma_start(wk_sb[:, 0:2], wk_re[:, 0:2])
        nc.sync.dma_start(xt0[:, 1], x3.ap().rearrange(
            "tc xi p g j t -> tc xi p g j t")[0, 1])
        nc.sync.dma_start(wk_sb[:, 2], wk_re[:, 2])
        csv0 = load_csv(0)
        wq_sb = pw.tile([128, 3, NPG, 2, QD], e4m3, tag="wq")
        wq_re = wq3.ap().rearrange("ti g p j m -> p ti g j m")
        for ti in range(3):
            nc.sync.dma_start(wq_sb[:, ti], wq_re[:, ti])
        eye_sb = pw.tile([128, 128], f16, tag="eye")
        nc.sync.dma_start(eye_sb[:], eyeI.ap()[:])
        wv_sb = pw.tile([128, 3, NPG, 2, D], e4m3, tag="wv")
        nc.sync.dma_start(wv_sb[:], wv3.ap().rearrange("ti g p j m -> p ti g j m"))
        # masks/wp are declared here but loaded after the first p1 chunk's
        # DMAs are queued (they are first used by attn/outproj of chunk 0)
        masks_sb = pw.tile([128, NM, WIDE], f16, tag="masks")
        wp_sb = pw.tile([128, REP, C], f16, tag="wp")

        def load_late_consts():
            nc.sync.dma_start(
                masks_sb[:], masksI.ap().rearrange("(nd p) t -> p nd t", p=128))
            nc.sync.dma_start(
                wp_sb[:], wpT.ap().rearrange("(qc p) c -> p qc c", p=128))

        # round-robin evacuation helper (spread PSUM->SBUF copies)
        _evac_rr = [0]

        def evac(dst, src, engines=("act", "dve", "pool")):
            e = engines[_evac_rr[0] % len(engines)]
            _evac_rr[0] += 1
            if e == "act":
                nc.scalar.copy(dst, src)
            elif e == "dve":
                nc.vector.tensor_copy(dst, src)
            else:
                nc.gpsimd.tensor_copy(dst, src)

        # ================= phase 1: one t-chunk of projections =================
        # terms: (x index, w index): xh*wh + xh*wl_u + xl16*wh16
        TERMS = [(0, 0), (0, 1), (1, 2)]

        def p1_parts(tci, xt, csv, prefetch=None):
            """Return a list of thunks emitting one t-chunk of phase 1; they
            are interleaved between the previous chunk's attention heads."""
            t0 = tci * TC
            cs, sn, ve_t = csv
            state = {}

            def do_proj(kind, m):
                acc = pacc.tile([128, TC], f32, tag="acc", name="acc")
                w_sb = {"q": wq_sb, "k": wk_sb, "v": wv_sb}[kind]
                msl = slice(m * D, (m + 1) * D) if kind == "q" else slice(0, D)
                for i, (xi, wi) in enumerate(TERMS):
                    for g in range(NPG):
                        nc.tensor.matmul(
                            acc[:], w_sb[:, wi, g, :, msl], xt[:, xi, g, :, :],
                            start=(i == 0 and g == 0),
                            stop=(i == 2 and g == NPG - 1),
                            perf_mode=PM.DoubleRow)
                return acc

            def rope_emit(src_n, src_w, dst, toff):
                ta = ptmp.tile([128, TC], f16, tag="t", name="ta")
                nc.vector.tensor_mul(ta[:], src_n, cs[:])
                tb = ptmp.tile([128, TC], f16, tag="t", name="tb")
                nc.vector.tensor_mul(tb[:], src_w, sn[:])
                nc.vector.tensor_add(dst[:, toff:toff + TC], ta[:], tb[:])

            def do_qk(kind, m, acc):
                qn_all = state["qn"]
                qraw = ptmp.tile([128, TC], f16, tag="qr", name="qraw")
                nc.scalar.activation(qraw[:], acc[:], AF.Copy, scale=1.0 / _WS)
                sqr = ptmp.tile([128, TC], f16, tag="t", name="sqr")
                nc.vector.tensor_mul(sqr[:], qraw[:], qraw[:])
                if kind == "k":
                    # transposed sumsq -> rk as columns, folded into exp scale
                    rkT = psR.tile([128, 4], f32, tag="row", name="rkT")
                    for j in range(TC // 128):
                        nc.tensor.matmul(rkT[:, j:j + 1],
                                         sqr[:, j * 128:(j + 1) * 128],
                                         ones_sb[:], start=True, stop=True)
                    lncol = ptmp.tile([128, 4], f32, tag="rc", name="lncol")
                    nc.scalar.activation(lncol[:], rkT[:], AF.Ln,
                                         scale=1.0 / D, bias=eps_row[:, 0:1])
                    nc.scalar.activation(rk_cols[:, tci * 4:tci * 4 + 4],
                                         lncol[:], AF.Exp, scale=-0.5)
                    # k's swap + rope immediately: kT gates the next chunk's
                    # first scores matmul, so don't wait for the q streams
                    ksw = ptmp.tile([128, TC], f16, tag="t", name="ksw")
                    nc.sync.dma_start(ksw[0:64], qraw[64:128])
                    nc.sync.dma_start(ksw[64:128], qraw[0:64])
                    rope_emit(qraw[:], ksw[:], kT_sb, t0)
                else:
                    # stats deferred so the Act queue is not blocked on the
                    # ss matmul, freeing the proj PSUM early
                    ss = psR.tile([1, TC], f32, tag="row", name="ss")
                    nc.tensor.matmul(ss[:], ones_sb[:], sqr[:],
                                     start=True, stop=True)
                    state[f"q{m}"] = (qraw, ss)

            def part_gate_k():
                if prefetch is not None:
                    prefetch()
                state["qn"] = pqn.tile([128, REP, TC], f16, tag="qn", name="qn")
                state["qsw"] = pqn.tile([128, REP, TC], f16, tag="qsw",
                                        name="qsw")
                zg = psR.tile([1, TC], f32, tag="row", name="zg")
                nc.tensor.matmul(zg[0:1, :], wg_sb[:],
                                 xt[0:VE_GATE_CH, 0, 0, 0, :],
                                 start=True, stop=True)
                ez = prow.tile([1, TC], f32, tag="g")
                nc.scalar.activation(ez[:], zg[:], AF.Exp, scale=-1.0 / 64.0)
                ez1 = prow.tile([1, TC], f32, tag="g")
                nc.vector.tensor_scalar_add(ez1[:], ez[:], 1.0)
                grow = prow.tile([1, TC], f16, tag="gh")
                nc.vector.reciprocal(grow[:], ez1[:])
                gbc = pbc.tile([128, TC], f16, tag="bc", name="gbc")
                nc.gpsimd.partition_broadcast(gbc[:], grow[:])
                state["gbc"] = gbc
                do_qk("k", 0, do_proj("k", 0))

            def part_q(m):
                def go():
                    do_qk("q", m, do_proj("q", m))
                return go

            def part_v():
                acc = do_proj("v", 0)
                vtmp = ptmp.tile([128, TC], f16, tag="t", name="vtmp")
                nc.vector.tensor_mul(vtmp[:], state["gbc"][:], ve_t[:])
                vfull = ptmp.tile([128, TC], f16, tag="t", name="vfull")
                nc.vector.scalar_tensor_tensor(
                    vfull[:], acc[:], 1.0 / _WS, vtmp[:],
                    mybir.AluOpType.mult, mybir.AluOpType.add)
                for j in range(TC // 128):
                    st = tci * (TC // 128) + j
                    vtr = psSY.tile([128, 128], f16, tag="sy", name="vtr")
                    nc.tensor.transpose(
                        vtr[:], vfull[:, j * 128:(j + 1) * 128], eye_sb[:])
                    evac(V_sb[:, st, :], vtr[:], ("dve", "act"))

            def stats_q(m):
                qn_all = state["qn"]
                qraw, ss = state[f"q{m}"]
                lnr = prow.tile([1, TC], f32, tag="r", name="lnr")
                nc.scalar.activation(lnr[:], ss[:], AF.Ln,
                                     scale=1.0 / D, bias=eps_row[0:1, :])
                rr = prow.tile([1, TC], f16, tag="rh", name="rr")
                nc.scalar.activation(rr[:], lnr[:], AF.Exp,
                                     scale=-0.5, bias=lncq_row[0:1, :])
                rbc = pbc.tile([128, TC], f16, tag="bc", name="rbc")
                nc.gpsimd.partition_broadcast(rbc[:], rr[:])
                nc.vector.tensor_mul(qn_all[:, m, :], rbc[:], qraw[:])

            def part_rope():
                qn_all, qsw_all = state["qn"], state["qsw"]
                nc.sync.dma_start(qsw_all[0:64], qn_all[64:128])
                nc.sync.dma_start(qsw_all[64:128], qn_all[0:64])
                for m in range(REP):
                    rope_emit(qn_all[:, m, :], qsw_all[:, m, :], qT_sb[m], t0)

            return [part_gate_k, part_q(0),
                    lambda: (part_q(1)(), stats_q(0)),
                    lambda: (part_q(2)(), stats_q(1)),
                    lambda: (part_v(), stats_q(2)),
                    part_rope]

        # ================= phase 2: attention + out-proj per chunk =============
        def attn_chunk(tci, fillers=()):
            t0 = tci * TC
            if win_finite:
                st_min = max(0, (t0 - window - 127) // 128 + 1)
            else:
                st_min = 0
            st_max = (t0 + TC - 1) // 128
            sts = list(range(st_min, st_max + 1))
            # put a fully-valid tile first when one exists: its full-width
            # start=True zeroes the yU bank without widening a masked tile
            def full_valid(st):
                delta = t0 - st * 128
                return (delta >= 128 and
                        (not win_finite or delta + TC - 1 <= window))
            fv = [st for st in sts if full_valid(st)]
            for i, st in enumerate(fv[:2]):
                sts.remove(st)
                sts.insert(i, st)
            nden = 2 if len(fv) >= 2 else 1
            fillers = list(fillers)

            for h in range(REP):
                if fillers:
                    fillers.pop(0)()
                yU = psSY.tile([128, TC], f32, tag="sy", name="yU")
                dens = [pden.tile([128, TC], f16, tag="d", name="den")]
                if nden == 2:
                    den2 = pden.tile([128, TC], f16, tag="d2", name="den2")
                    dens.append(den2)
                for idx, st in enumerate(sts):
                    s0 = st * 128
                    delta = t0 - s0
                    causal_p = delta <= 0
                    window_p = win_finite and delta > window - (TC - 1)
                    v0 = max(0, -delta) if causal_p else 0
                    v1 = min(TC, window - delta + 128) if window_p else TC
                    if idx == 0:
                        # full width so the yU start=True zeroes the whole
                        # bank; mask matmuls cover the out-of-range columns
                        # (the mask tiles encode the exact condition)
                        v0, v1 = 0, TC
                    nmm = int(causal_p) + int(window_p)
                    sc = psSY.tile([128, TC], f32, tag="sy", name="sc")
                    nc.tensor.matmul(sc[:, v0:v1], kT_sb[:, s0:s0 + 128],
                                     qT_sb[h][:, t0 + v0:t0 + v1],
                                     start=True, stop=(nmm == 0))
                    if causal_p:    # masked cols [v0, 128-delta)
                        c0, c1 = v0, min(v1, 128 - delta)
                        off = delta + (TC - 128)
                        nmm -= 1
                        nc.tensor.matmul(sc[:, c0:c1], eye_sb[:],
                                         masks_sb[:, 0, off + c0:off + c1],
                                         start=False, stop=(nmm == 0))
                    if window_p:    # masked cols suffix
                        c0 = max(v0, (window - delta + 1) // 128 * 128)
                        c1 = v1
                        off = delta - wmin
                        nmm -= 1
                        nc.tensor.matmul(sc[:, c0:c1], eye_sb[:],
                                         masks_sb[:, 1, off + c0:off + c1],
                                         start=False, stop=(nmm == 0))
                    P = pP.tile([128, TC], f16, tag="P", name="P")
                    nc.scalar.activation(P[:, v0:v1], sc[:, v0:v1], AF.Exp,
                                         scale=rk_cols[:, st:st + 1],
                                         bias=pshift_col[:, 0:1])
                    den = dens[idx % nden]
                    if idx < nden:  # full width: replaces a memset
                        nc.vector.tensor_copy(den[:], P[:])
                    else:
                        nc.vector.tensor_add(den[:, v0:v1], den[:, v0:v1],
                                             P[:, v0:v1])
                    nc.tensor.matmul(yU[:, v0:v1], V_sb[:, st, :],
                                     P[:, v0:v1], start=(idx == 0),
                                     stop=(idx == len(sts) - 1))
                drow = psR.tile([1, TC], f32, tag="row", name="dr")
                for di in range(nden):
                    nc.tensor.matmul(drow[:], ones_sb[:], dens[di][:],
                                     start=(di == 0), stop=(di == nden - 1))
                dinv = prow.tile([1, TC], f16, tag="rh", name="dinv")
                nc.vector.reciprocal(dinv[:], drow[:])
                dbc = pbc.tile([128, TC], f16, tag="bc", name="dbc")
                nc.gpsimd.partition_broadcast(dbc[:], dinv[:])
                nc.vector.tensor_mul(yT_sb[h][:, t0:t0 + TC], dbc[:], yU[:])

            # ---- out-proj for this chunk ----
            # last chunk: finer DMA granularity to shrink the drain tail
            ngrp = 6 if tci == NTC - 1 else 2
            npc = NCC // ngrp
            for grp in range(ngrp):
                if fillers:
                    fillers.pop(0)()
                ot = pout.tile([128, npc, TC], f16, tag="ot", name="ot")
                for ci in range(npc):
                    cc = grp * npc + ci
                    o = psSY.tile([128, TC], f32, tag="sy", name="o")
                    for m in range(REP):
                        nc.tensor.matmul(
                            o[:], wp_sb[:, m, cc * 128:(cc + 1) * 128],
                            yT_sb[m][:, t0:t0 + TC],
                            start=(m == 0), stop=(m == REP - 1))
                    evac(ot[:, ci, :], o[:], ("act", "dve"))
                nc.sync.dma_start(
                    outT.ap().rearrange("(cc p) t -> p cc t", p=128)
                    [:, grp * npc:grp * npc + npc, t0:t0 + TC], ot[:])
            for f in fillers:
                f()

        # ================= interleaved schedule =================
        with nc.allow_low_precision(reason="fp16 validated vs numpy: 2e-3 max rel"):
            xt1 = load_x(1)
            csv1 = load_csv(1)
            xts = {0: xt0, 1: xt1}
            csvs = {0: csv0, 1: csv1}
            for p in p1_parts(0, xt0, csv0):
                p()
            load_late_consts()
            for tci in range(NTC):
                fillers = ()
                if tci + 1 < NTC:
                    prefetch = None
                    if tci + 2 < NTC:
                        def prefetch(t2=tci + 2):
                            xts[t2] = load_x(t2)
                            csvs[t2] = load_csv(t2)
                    fillers = p1_parts(tci + 1, xts[tci + 1], csvs[tci + 1],
                                       prefetch)
                attn_chunk(tci, fillers)

    nc.compile()
    nc._mask_cfg = {"wide": WIDE, "cmin": -(TC - 128), "wmin": wmin}
    return nc


def _qx_hilo(a):
    """x fp8 pair: a ~= hi + lo16/16."""
    import ml_dtypes
    hi = a.astype(ml_dtypes.float8_e4m3fn)
    hif = hi.astype(np.float32)
    lo16 = ((a - hif) * 16.0).astype(ml_dtypes.float8_e4m3fn)
    return hi, lo16


def _qw_tri(a):
    """weight fp8 triplet (prescaled): hi, lo_unscaled, hi/16."""
    import ml_dtypes
    hi = a.astype(ml_dtypes.float8_e4m3fn)
    hif = hi.astype(np.float32)
    lo_u = (a - hif).astype(ml_dtypes.float8_e4m3fn)
    hi16 = (hif / 16.0).astype(ml_dtypes.float8_e4m3fn)
    return hi, lo_u, hi16


def _pack_pairs(a3, outdim_last):
    """[3][rows(C), cols] -> [3, NPG, 128, 2, cols] pair-grouped layout."""
    out = np.stack([t.reshape(NPG, 2, 128, -1).transpose(0, 2, 1, 3)
                    for t in a3], axis=0)
    return np.ascontiguousarray(out)


def _prep_inputs(nc, window, x, ve, cos, sin, Wq, Wk, Wv, Wproj, Wg):
    import ml_dtypes
    f16t = ml_dtypes.float16 if hasattr(ml_dtypes, 'float16') else np.float16
    win_finite = 0 <= window < T
    cosT = np.ascontiguousarray(cos.reshape(T, D // 2).T)
    sinT = np.ascontiguousarray(sin.reshape(T, D // 2).T)
    cos2 = np.concatenate([cosT, cosT], axis=0).astype(np.float16)
    sin2m = np.concatenate([sinT, -sinT], axis=0).astype(np.float16)
    eye = np.eye(128, dtype=np.float16)
    ones = np.ones((128, 1), dtype=np.float16)

    ds = np.arange(128)[:, None]
    wcfg = nc._mask_cfg
    j = np.arange(wcfg["wide"])[None, :]
    mc = np.where(j + wcfg["cmin"] - ds >= 0, 0.0, _MASKVAL).astype(np.float16)
    rows = [mc]
    if win_finite:
        mw = np.where(j + wcfg["wmin"] - ds <= window, 0.0,
                      _MASKVAL).astype(np.float16)
        rows.append(mw)
    masks = np.concatenate(rows, axis=0)

    # x pairs per batch, chunk-major: [NTC, 2, 128, NPG, 2, TC]
    x3b = []
    for b in range(B):
        xT = x[b].T                                   # [C, T]
        tri = np.stack(_qx_hilo(xT), axis=0)          # [2, C, T]
        tri = tri.reshape(2, NPG, 2, 128, T).transpose(0, 3, 1, 2, 4)
        chunks = [tri[:, :, :, :, t0:t0 + TC] for t0 in range(0, T, TC)]
        x3b.append(np.ascontiguousarray(np.stack(chunks, axis=0)))

    in_maps = []
    for core in range(N_CORES):
        b, g = divmod(core, KV)
        sl_q = slice(g * QD, (g + 1) * QD)
        sl_d = slice(g * D, (g + 1) * D)
        wq_t = _pack_pairs(list(_qw_tri(Wq[sl_q].T * _WS)), QD)
        wk_t = _pack_pairs(list(_qw_tri(Wk[sl_d].T * _WS)), D)
        wv_t = _pack_pairs(list(_qw_tri(Wv[sl_d].T * _WS)), D)
        in_maps.append({
            "x3": x3b[b],
            "wq3": wq_t, "wk3": wk_t, "wv3": wv_t,
            "wpT": np.ascontiguousarray(Wproj[:, sl_q].T).astype(np.float16),
            "wg": np.ascontiguousarray(
                (Wg[g] * 64.0).reshape(VE_GATE_CH, 1)).astype(
                    __import__('ml_dtypes').float8_e4m3fn),
            "veT": np.ascontiguousarray(3.0 * ve[b, :, sl_d].T).astype(np.float16),
            "cos2": cos2, "sin2m": sin2m, "eyeI": eye, "onesI": ones,
            "masksI": masks,
        })
    return in_maps


def kernel(x, ve, cos, sin, Wq, Wk, Wv, Wproj, Wg, window, _trace=False):
    window = int(window)
    if window not in _CACHE:
        _CACHE[window] = _build(window)
    nc = _CACHE[window]

    in_maps = _prep_inputs(nc, window,
                           np.asarray(x, np.float32), np.asarray(ve, np.float32),
                           np.asarray(cos, np.float32), np.asarray(sin, np.float32),
                           np.asarray(Wq, np.float32), np.asarray(Wk, np.float32),
                           np.asarray(Wv, np.float32), np.asarray(Wproj, np.float32),
                           np.asarray(Wg, np.float32))

    res = run_bass_kernel_spmd(nc, in_maps, core_ids=list(range(N_CORES)),
                               trace=_trace)

    out = np.empty((B, T, C), dtype=np.float32)
    for b in range(B):
        acc = res.results[b * KV]["outT"].astype(np.float32)
        for g in range(1, KV):
            acc += res.results[b * KV + g]["outT"].astype(np.float32)
        out[b] = acc.T
    if _trace:
        kernel._last_trace = res
    return out
